# revision 17
# baseline (speedup 1.0000x reference)
"""GQA attention block (B=1, T=2048, HID=2048, NQ=16, NKV=8, D=128) on 8 TRN2
NeuronCores.

Sharding: tensor-parallel over heads. Core c owns q-heads {2c, 2c+1} and
kv-head c. The 8 partial outputs are summed on the host (scaled 1/(VS*BETA)).

v2 speed strategy (validated against the TimelineSim cost model + f64 ref):
  - projections: 3-term split-fp8 (xh*wh + xl*wh + xh*wl) with K=256
    DoubleRow matmuls (0.5 cyc/row in the cost model). Wq/Wk pre-scaled by
    WS=64 (cancels through RMS norm), Wv by VS=32.
  - V is projected directly transposed ([t, d] tiles, stationary = x chunk)
    so no PE transposes / identity are needed.
  - q/k: RMS-norm + RoPE on DVE in bf16; Act reads raw projections straight
    from PSUM (no staging copy).
  - attention: at = exp(score/sqrt(D) - 2) fp8 for q-rows >= 512, bf16 for
    the first 512 rows. Causal handling is fine-grained on the diagonal
    512x512 block: per 128-query subtile only the needed key tiles are
    computed, and only the true-diagonal 128x128 tile is min-masked
    (mask in {0, 240}: min(sat, 0) = 0 kills acausal fp8-overflowed exp).
  - denominators: ones-stationary matmuls accumulated alongside ctx (PSUM).
  - o_proj: 3-term split-fp8 DoubleRow with BOTH heads packed into K=256
    (ctx split hi/lo on DVE/Pool; Wo pre-scaled by BETA=64, ctx carries
    VS=32; host divides by 2048). Output rows for q-blocks {0,2,3} are
    DMA'd f32 straight from PSUM (no copy); the last-processed block (1)
    goes through bf16 copies for a short tail.
  - schedule: q-blocks processed in order 0,3,2,1; each block's o_proj
    tiles are interleaved as PE filler into the next block's attention
    (which is Act-exp paced), keeping the PE queue dense.
"""

import sys

sys.path.insert(0, "/opt/trn_rl_repo")

import numpy as np
import ml_dtypes

import concourse.bass as bass  # noqa: F401  (bass must import before tile)
import concourse.mybir as mybir
import concourse.tile as tile
from concourse import bacc
from concourse.bass_utils import run_bass_kernel_spmd

N_CORES = 8
T = 2048
HID = 2048
NQ, NKV, D = 16, 8, 128
HQ = NQ // N_CORES  # q heads per core = 2
EPS = 1e-6
SCALE = D**-0.5
SHIFT = 2.0
WS = 64.0   # Wq/Wk pre-scale (cancels in RMS norm)
VS = 32.0   # Wv pre-scale == ctx scale alpha (fp8 range)
BETA = 64.0  # Wo pre-scale (fp8 range); host divides by VS*BETA

P = 128
H = D // 2
KP = HID // 256     # 8 K-pair chunks of 256
NTR = T // 512      # 4 T-ranges of 512

F32 = mybir.dt.float32
BF16 = mybir.dt.bfloat16
F8 = mybir.dt.float8e4
DR = mybir.MatmulPerfMode.DoubleRow
ACT_EXP = mybir.ActivationFunctionType.Exp
ACT_LN = mybir.ActivationFunctionType.Ln
ACT_SQUARE = mybir.ActivationFunctionType.Square
MIN = mybir.AluOpType.min
MULT = mybir.AluOpType.mult
SUB = mybir.AluOpType.subtract

QR_ORDER = [0, 3, 2, 1]  # last one takes the bf16-copy output path


def build_nc():
    nc = bacc.Bacc("TRN2", target_bir_lowering=False, debug=False,
                   num_devices=N_CORES)

    # ---- DRAM tensors (names = in_map keys) ----
    xh = nc.dram_tensor("xh", [P, KP, 2, T], F8, kind="ExternalInput")
    xl = nc.dram_tensor("xl", [P, KP, 2, T], F8, kind="ExternalInput")
    wqh = nc.dram_tensor("wqh", [P, KP, 2, HQ * D], F8, kind="ExternalInput")
    wql = nc.dram_tensor("wql", [P, KP, 2, HQ * D], F8, kind="ExternalInput")
    wkh = nc.dram_tensor("wkh", [P, KP, 2, D], F8, kind="ExternalInput")
    wkl = nc.dram_tensor("wkl", [P, KP, 2, D], F8, kind="ExternalInput")
    wvh = nc.dram_tensor("wvh", [P, KP, 2, D], F8, kind="ExternalInput")
    wvl = nc.dram_tensor("wvl", [P, KP, 2, D], F8, kind="ExternalInput")
    woh = nc.dram_tensor("woh", [P, HQ, HID], F8, kind="ExternalInput")
    wol = nc.dram_tensor("wol", [P, HQ, HID], F8, kind="ExternalInput")
    cosT = nc.dram_tensor("cosT", [P, T], BF16, kind="ExternalInput")
    sinT = nc.dram_tensor("sinT", [P, T], BF16, kind="ExternalInput")
    qw = nc.dram_tensor("qw", [P, 1], F32, kind="ExternalInput")
    kw = nc.dram_tensor("kw", [P, 1], F32, kind="ExternalInput")
    masks = nc.dram_tensor("masks", [P, P], BF16, kind="ExternalInput")
    out = nc.dram_tensor("out", [T, HID], BF16, kind="ExternalOutput")

    with tile.TileContext(nc) as tc:
        with (
            tc.tile_pool(name="cst", bufs=1) as cst,
            tc.tile_pool(name="fin", bufs=1) as fin,
        ):
            # ---------- constants / weights resident in SBUF ----------
            wqh_sb = cst.tile([P, KP, 2, HQ * D], F8)
            wql_sb = cst.tile([P, KP, 2, HQ * D], F8)
            wkh_sb = cst.tile([P, KP, 2, D], F8)
            wkl_sb = cst.tile([P, KP, 2, D], F8)
            wvh_sb = cst.tile([P, KP, 2, D], F8)
            wvl_sb = cst.tile([P, KP, 2, D], F8)
            woh_sb = cst.tile([P, HQ, HID], F8)
            wol_sb = cst.tile([P, HQ, HID], F8)
            masks_sb = cst.tile([P, P], BF16)
            cos_sb = cst.tile([P, T], BF16)
            sin_sb = cst.tile([P, T], BF16)
            qw_sb = cst.tile([P, 1], F32)
            kw_sb = cst.tile([P, 1], F32)
            nc.gpsimd.dma_start(qw_sb[:], qw[:])
            nc.gpsimd.dma_start(kw_sb[:], kw[:])
            ones_b = cst.tile([P, 1], BF16)
            nc.vector.memset(ones_b[:], 1.0)
            # DoubleRow ldweights requires the 2-plane dim step % 16 == 0
            w1_8 = cst.tile([P, 2, 16], F8)
            nc.vector.memset(w1_8[:], 1.0)
            shift_sb = cst.tile([P, 1], F32)
            nc.vector.memset(shift_sb[:], -SHIFT)

            # post RMS+RoPE q/k in bf16 (d on partitions)
            qT = [fin.tile([P, T], BF16, name=f"qT_{s}") for s in range(3)]
            # V (VS x): fp8 plane-pairs (plane = st parity) + bf16 st 0-3
            vp = fin.tile([P, T // 256, 2, D], F8)
            v0b = fin.tile([P, 4, D], BF16)
            # normalized ctx (VS x), fp8 hi/lo, plane = head
            ctxC = fin.tile([P, HQ, T], F8)
            ctxL = fin.tile([P, HQ, T], F8)

            # ==== Phase A (split-fp8 DR projections) + B (RMS+RoPE) ====
            with (
                tc.tile_pool(name="xp", bufs=4) as xp,
                tc.tile_pool(name="tmpp", bufs=6) as tmpp,
                tc.tile_pool(name="psA", bufs=4, space="PSUM") as psA,
                tc.tile_pool(name="psV", bufs=2, space="PSUM") as psV,
                tc.tile_pool(name="psB", bufs=2, space="PSUM") as psB,
            ):
                for tr in range(NTR):
                    ts = slice(tr * 512, (tr + 1) * 512)
                    xhc = xp.tile([P, KP, 2, 512], F8, name="xhc")
                    xlc = xp.tile([P, KP, 2, 512], F8, name="xlc")
                    if tr == 0:
                        # ordered for fastest PE start: wq-hi + x-hi first,
                        # spread across SP/Act/DVE queues (SEQ serializes
                        # per queue, transfers serialize on DMA_ENGINES)
                        nc.sync.dma_start(wqh_sb[:], wqh[:])
                        nc.scalar.dma_start(xhc[:, 0:4, :, :], xh[:, 0:4, :, ts])
                        nc.sync.dma_start(wkh_sb[:], wkh[:])
                        nc.sync.dma_start(wvh_sb[:], wvh[:])
                        nc.scalar.dma_start(xhc[:, 4:8, :, :], xh[:, 4:8, :, ts])
                        nc.scalar.dma_start(xlc[:, 0:4, :, :], xl[:, 0:4, :, ts])
                        nc.scalar.dma_start(xlc[:, 4:8, :, :], xl[:, 4:8, :, ts])
                        nc.gpsimd.dma_start(wql_sb[:], wql[:])
                        nc.gpsimd.dma_start(wkl_sb[:], wkl[:])
                        nc.gpsimd.dma_start(wvl_sb[:], wvl[:])
                        nc.scalar.dma_start(cos_sb[:], cosT[:])
                        nc.scalar.dma_start(sin_sb[:], sinT[:])
                        nc.gpsimd.dma_start(masks_sb[:], masks[:])
                    else:
                        nc.sync.dma_start(xhc[:], xh[:, :, :, ts])
                        nc.sync.dma_start(xlc[:], xl[:, :, :, ts])
                    if tr == 1:
                        nc.gpsimd.dma_start(woh_sb[:], woh[:])
                        nc.gpsimd.dma_start(wol_sb[:], wol[:])

                    terms = ((wqh_sb, wkh_sb, wvh_sb, xhc),
                             (wql_sb, wkl_sb, wvl_sb, xhc),
                             (wqh_sb, wkh_sb, wvh_sb, xlc))

                    # --- projections q0, q1, k (d on partitions) ---
                    raw = []
                    for s in range(3):
                        ps = psA.tile([P, 512], F32, name="psA_t")
                        n = 3 * KP
                        i = 0
                        for wq_t, wk_t, _, xt_ in terms:
                            wt = wq_t if s < 2 else wk_t
                            cs = slice(s * D, (s + 1) * D) if s < 2 \
                                else slice(0, D)
                            for kp in range(KP):
                                nc.tensor.matmul(
                                    ps[:], wt[:, kp, :, cs], xt_[:, kp, :, :],
                                    perf_mode=DR,
                                    start=(i == 0), stop=(i == n - 1),
                                )
                                i += 1
                        raw.append(ps)

                    # --- v: projected directly transposed into [t, d] ---
                    psv = psV.tile([P, 4, D], F32, name="psv")
                    n = 3 * KP
                    i = 0
                    for _, _, wv_t, xt_ in terms:
                        for kp in range(KP):
                            for j in range(4):
                                jts = slice(j * P, (j + 1) * P)
                                nc.tensor.matmul(
                                    psv[:, j, :], xt_[:, kp, :, jts],
                                    wv_t[:, kp, :, :],
                                    perf_mode=DR,
                                    start=(i == 0), stop=(i == n - 1),
                                )
                            i += 1
                    for j in range(4):
                        st = 4 * tr + j
                        nc.vector.tensor_copy(vp[:, st // 2, st % 2, :],
                                              psv[:, j, :])
                        if tr == 0:
                            nc.gpsimd.tensor_copy(v0b[:, st, :], psv[:, j, :])

                    # --- B: RMS norm + RoPE for q0, q1, k (bf16) ---
                    for s in range(3):
                        w_sb = qw_sb if s < 2 else kw_sb
                        # free the psA bank early for the next projection /
                        # the phase-C PSUM pools (Pool has slack here)
                        src = tmpp.tile([P, 512], F32, name="src")
                        nc.gpsimd.tensor_copy(src[:], raw[s][:])
                        sq = tmpp.tile([P, 512], BF16, name="sq")
                        nc.scalar.activation(sq[:], src[:], ACT_SQUARE)
                        ssum = psB.tile([1, 512], F32, name="ssum")
                        nc.tensor.matmul(ssum[:], ones_b[:], sq[:],
                                         start=True, stop=True)
                        # src holds 64*q; host folds sqrt(D) into q/k norm
                        # weights, eps is negligible vs ssum ~ 3e5:
                        # rinv = ssum^-0.5 via exp(-0.5*ln(ssum)) -- keeps
                        # every Act func in one table set (no mid-kernel
                        # LoadActFuncSet switches)
                        lnv = tmpp.tile([1, 512], F32, name="lnv")
                        nc.scalar.activation(lnv[:], ssum[:], ACT_LN)
                        rinv = tmpp.tile([1, 512], F32, name="rinv")
                        nc.scalar.activation(rinv[:], lnv[:], ACT_EXP,
                                             scale=-0.5)
                        rb = tmpp.tile([P, 512], F32, name="rb")
                        nc.gpsimd.partition_broadcast(rb[:], rinv[:])
                        nq = tmpp.tile([P, 512], BF16, name="nq")
                        nc.vector.scalar_tensor_tensor(
                            nq[:], src[:], w_sb[:], rb[:], MULT, MULT,
                        )
                        # RoPE: sin pre-rolled by 64 partitions with the
                        # rotate-half sign folded in; one full-width add.
                        psn = tmpp.tile([P, 512], BF16, name="psn")
                        nc.vector.tensor_mul(psn[0:H, :], nq[H:D, :],
                                             sin_sb[H:D, ts])
                        nc.vector.tensor_mul(psn[H:D, :], nq[0:H, :],
                                             sin_sb[0:H, ts])
                        pc = tmpp.tile([P, 512], BF16, name="pc")
                        nc.vector.tensor_mul(pc[:], nq[:], cos_sb[:, ts])
                        nc.vector.tensor_add(qT[s][:, ts], pc[:], psn[:])

            # ===== Phase C: attention + o_proj =====
            with (
                tc.tile_pool(name="atp", bufs=5) as atp,
                tc.tile_pool(name="adp", bufs=3) as adp,
                tc.tile_pool(name="cfp", bufs=2) as cfp,
                tc.tile_pool(name="otp", bufs=4) as otp,
                tc.tile_pool(name="attp", bufs=4) as attp,
                tc.tile_pool(name="psP", bufs=2, space="PSUM") as psP,
                tc.tile_pool(name="psCX", bufs=2, space="PSUM") as psCX,
                tc.tile_pool(name="psSM", bufs=1, space="PSUM") as psSM,
                tc.tile_pool(name="psD", bufs=3, space="PSUM") as psD,
            ):
                kT = qT[2]
                pending = []

                def make_task(qr, tt, nr, idx):
                    abs_tt = 4 * qr + tt
                    tts = slice(abs_tt * P, (abs_tt + 1) * P)
                    ns = slice(nr * 512, (nr + 1) * 512)

                    def go():
                        ps = psD.tile([P, 512], F32, name="psD_t")
                        for i, (cs, ws) in enumerate(
                                ((ctxC, woh_sb), (ctxL, woh_sb),
                                 (ctxC, wol_sb))):
                            nc.tensor.matmul(
                                ps[:], cs[:, :, tts], ws[:, :, ns],
                                perf_mode=DR,
                                start=(i == 0), stop=(i == 2))
                        ot = otp.tile([P, 512], BF16, name="ot")
                        r = idx % 3
                        if r == 0:
                            nc.vector.tensor_copy(ot[:], ps[:])
                        elif r == 1:
                            nc.scalar.copy(ot[:], ps[:])
                        else:
                            nc.gpsimd.tensor_copy(ot[:], ps[:])
                        # out DMAs only on sync: a dma_start blocks its
                        # issuing engine's SEQ until the copy dependency
                        # resolves, so compute queues must not carry them
                        nc.sync.dma_start(out[tts, ns], ot[:])
                    return go

                def emit_fill(k):
                    for _ in range(min(k, len(pending))):
                        pending.pop(0)()

                for qi, qr in enumerate(QR_ORDER):
                    qs = slice(qr * 512, (qr + 1) * 512)
                    at_dt = BF16 if qr == 0 else F8
                    for h in range(HQ):
                        n_off = 2 * qr
                        ctx_ps = psCX.tile([P, 512], F32, name="ctx_ps")
                        sums_ps = psSM.tile([1, 512], F32, name="sums_ps")
                        # --- fully-causal pairs below the diagonal block ---
                        for pi in range(n_off):
                            at = atp.tile([P, 2, 512], at_dt, name="at")
                            for half in range(2):
                                st = 2 * pi + half
                                s_ps = psP.tile([P, 512], F32, name="s_t")
                                nc.tensor.matmul(
                                    s_ps[:], kT[:, st * P:(st + 1) * P],
                                    qT[h][:, qs], start=True, stop=True)
                                nc.scalar.activation(
                                    at[:, half, :], s_ps[:], ACT_EXP,
                                    scale=SCALE, bias=shift_sb[:])
                            nc.tensor.matmul(
                                ctx_ps[:], vp[:, pi, :, :], at[:],
                                perf_mode=DR,
                                start=(pi == 0), stop=False)
                            nc.tensor.matmul(
                                sums_ps[:], w1_8[:, :, 0:1], at[:],
                                perf_mode=DR,
                                start=(pi == 0), stop=False)
                            emit_fill(1)
                        # --- diagonal 512x512 block, 128-query granular ---
                        for j in range(4):
                            jsl = slice(j * P, (j + 1) * P)
                            qjs = slice(qr * 512 + j * P,
                                        qr * 512 + (j + 1) * P)
                            sd = psP.tile([P, 4, P], F32, name="s_t")
                            for i in range(j + 1):
                                st = 4 * qr + i
                                nc.tensor.matmul(
                                    sd[:, i, :], kT[:, st * P:(st + 1) * P],
                                    qT[h][:, qjs], start=True, stop=True)
                            ad = adp.tile([P, 4, P], at_dt, name="ad")
                            nc.scalar.activation(
                                ad[:, 0:j + 1, :], sd[:, 0:j + 1, :],
                                ACT_EXP, scale=SCALE, bias=shift_sb[:])
                            # only the true-diagonal tile needs masking
                            nc.vector.tensor_tensor(
                                ad[:, j, :], ad[:, j, :], masks_sb[:], MIN)
                            if qr == 0:
                                for i in range(j + 1):
                                    nc.tensor.matmul(
                                        ctx_ps[:, jsl], v0b[:, i, :],
                                        ad[:, i, :],
                                        start=(i == 0), stop=(i == j))
                                    nc.tensor.matmul(
                                        sums_ps[0:1, jsl], ones_b[:],
                                        ad[:, i, :],
                                        start=(i == 0), stop=(i == j))
                            else:
                                np_full = (j + 1) // 2
                                for p_ in range(np_full):
                                    last = (j % 2 == 1) and (p_ == np_full - 1)
                                    nc.tensor.matmul(
                                        ctx_ps[:, jsl],
                                        vp[:, 2 * qr + p_, :, :],
                                        ad[:, 2 * p_:2 * p_ + 2, :],
                                        perf_mode=DR,
                                        start=False, stop=last)
                                    nc.tensor.matmul(
                                        sums_ps[0:1, jsl], w1_8[:, :, 0:1],
                                        ad[:, 2 * p_:2 * p_ + 2, :],
                                        perf_mode=DR,
                                        start=False, stop=last)
                                if j % 2 == 0:  # odd plane count: tail tile
                                    nc.tensor.matmul(
                                        ctx_ps[:, jsl],
                                        vp[:, 2 * qr + j // 2, j % 2, :],
                                        ad[:, j, :],
                                        start=False, stop=True)
                                    nc.tensor.matmul(
                                        sums_ps[0:1, jsl], w1_8[:, 0, 0:1],
                                        ad[:, j, :],
                                        start=False, stop=True)
                            emit_fill(1)
                        # --- normalize + fp8 hi/lo split of ctx ---
                        recip = attp.tile([1, 512], F32, name="recip")
                        nc.vector.reciprocal_approx_fast(recip[:], sums_ps[:])
                        rb = attp.tile([P, 512], F32, name="rbc")
                        nc.gpsimd.partition_broadcast(rb[:], recip[:])
                        cf = cfp.tile([P, 512], F32, name="cf")
                        nc.vector.tensor_mul(cf[:], ctx_ps[:], rb[:])
                        nc.gpsimd.tensor_copy(ctxC[:, h, qs], cf[:])
                        nc.vector.scalar_tensor_tensor(
                            ctxL[:, h, qs], cf[:], 1.0, ctxC[:, h, qs],
                            MULT, SUB)
                        emit_fill(1)
                    # queue this block's o_proj tiles as PE filler
                    for tt in range(4):
                        for nr in range(4):
                            pending.append(
                                make_task(qr, tt, nr, 4 * tt + nr))
                    if qi == len(QR_ORDER) - 1:
                        emit_fill(len(pending))

    nc.compile()
    return nc


_NC_CACHE = None


def get_nc():
    global _NC_CACHE
    if _NC_CACHE is None:
        _NC_CACHE = build_nc()
    return _NC_CACHE


F8NP = ml_dtypes.float8_e4m3
BF16NP = ml_dtypes.bfloat16


def _fold_hid(a):
    """[HID, C] -> [P, KP, 2, C] with hid = kp*256 + pl*128 + p."""
    c = a.shape[1]
    return np.ascontiguousarray(
        a.reshape(KP, 2, P, c).transpose(2, 0, 1, 3))


def _split8(a):
    hi = a.astype(F8NP)
    lo = (a - hi.astype(np.float32)).astype(F8NP)
    return hi, lo


def make_in_maps(x, cos, sin, Wq, Wk, Wv, Wo, q_norm_w, k_norm_w):
    x = np.asarray(x, dtype=np.float32).reshape(T, HID)
    xf = _fold_hid(np.ascontiguousarray(x.T).reshape(HID, T))
    xh, xl = _split8(xf)
    cosb = np.ascontiguousarray(
        np.asarray(cos, np.float32).T).astype(BF16NP)
    # rolled by 64 with rotate-half signs folded in:
    # psn[0:64] (subtracted in ref) uses rows 64:128 -> negate those rows
    sr = np.roll(np.asarray(sin, np.float32).T, 64, axis=0)
    sr[64:, :] *= -1.0
    sinb = np.ascontiguousarray(sr).astype(BF16NP)
    # sqrt(D) folded here: kernel computes rinv = (sum (64 q)^2)^-0.5
    sqd = np.float32(np.sqrt(D))
    qwa = np.ascontiguousarray(
        np.asarray(q_norm_w, np.float32).reshape(D, 1) * sqd)
    kwa = np.ascontiguousarray(
        np.asarray(k_norm_w, np.float32).reshape(D, 1) * sqd)
    si = np.arange(P)[:, None]
    qi = np.arange(P)[None, :]
    masks = np.where(si <= qi, 240.0, 0.0).astype(BF16NP)
    Wq = np.asarray(Wq, np.float32) * WS
    Wk = np.asarray(Wk, np.float32) * WS
    Wv = np.asarray(Wv, np.float32) * VS
    Wo = np.asarray(Wo, np.float32) * BETA
    in_maps = []
    for c in range(N_CORES):
        wqh_, wql_ = _split8(_fold_hid(Wq[:, c * HQ * D:(c + 1) * HQ * D]))
        wkh_, wkl_ = _split8(_fold_hid(Wk[:, c * D:(c + 1) * D]))
        wvh_, wvl_ = _split8(_fold_hid(Wv[:, c * D:(c + 1) * D]))
        wo_ = np.ascontiguousarray(
            Wo[c * HQ * D:(c + 1) * HQ * D, :].reshape(HQ, P, HID)
            .transpose(1, 0, 2))
        woh_, wol_ = _split8(wo_)
        in_maps.append({
            "xh": xh, "xl": xl,
            "wqh": wqh_, "wql": wql_,
            "wkh": wkh_, "wkl": wkl_,
            "wvh": wvh_, "wvl": wvl_,
            "woh": woh_, "wol": wol_,
            "cosT": cosb, "sinT": sinb,
            "qw": qwa, "kw": kwa,
            "masks": masks,
        })
    return in_maps


def kernel(x, cos, sin, Wq, Wk, Wv, Wo, q_norm_w, k_norm_w):
    nc = get_nc()
    in_maps = make_in_maps(x, cos, sin, Wq, Wk, Wv, Wo, q_norm_w, k_norm_w)
    res = run_bass_kernel_spmd(nc, in_maps, core_ids=list(range(N_CORES)))
    acc = np.zeros((T, HID), dtype=np.float32)
    for c in range(N_CORES):
        acc += np.asarray(res.results[c]["out"], np.float32)
    acc *= 1.0 / (VS * BETA)
    return acc.reshape(1, T, HID)


# revision 46
# speedup vs baseline: 1.0563x; 1.0563x over previous
"""GQA attention block (B=1, T=2048, HID=2048, NQ=16, NKV=8, D=128) on 8 TRN2
NeuronCores.

Sharding: tensor-parallel over heads. Core c owns q-heads {2c, 2c+1} and
kv-head c. The 8 partial outputs are summed on the host (scaled 1/(VS*BETA)).

v2 speed strategy (validated against the TimelineSim cost model + f64 ref):
  - projections: 3-term split-fp8 (xh*wh + xl*wh + xh*wl) with K=256
    DoubleRow matmuls (0.5 cyc/row in the cost model). Wq/Wk pre-scaled by
    WS=64 (cancels through RMS norm), Wv by VS=32.
  - V is projected directly transposed ([t, d] tiles, stationary = x chunk)
    so no PE transposes / identity are needed.
  - q/k: RMS-norm + RoPE on DVE in bf16; Act reads raw projections straight
    from PSUM (no staging copy).
  - attention: at = exp(score/sqrt(D) - 2) fp8 for q-rows >= 512, bf16 for
    the first 512 rows. Causal handling is fine-grained on the diagonal
    512x512 block: per 128-query subtile only the needed key tiles are
    computed, and only the true-diagonal 128x128 tile is min-masked
    (mask in {0, 240}: min(sat, 0) = 0 kills acausal fp8-overflowed exp).
  - denominators: ones-stationary matmuls accumulated alongside ctx (PSUM).
  - o_proj: 3-term split-fp8 DoubleRow with BOTH heads packed into K=256
    (ctx split hi/lo on DVE/Pool; Wo pre-scaled by BETA=64, ctx carries
    VS=32; host divides by 2048). Output rows for q-blocks {0,2,3} are
    DMA'd f32 straight from PSUM (no copy); the last-processed block (1)
    goes through bf16 copies for a short tail.
  - schedule: q-blocks processed in order 0,3,2,1; each block's o_proj
    tiles are interleaved as PE filler into the next block's attention
    (which is Act-exp paced), keeping the PE queue dense.
"""

import sys

sys.path.insert(0, "/opt/trn_rl_repo")

import numpy as np
import ml_dtypes

import concourse.bass as bass  # noqa: F401  (bass must import before tile)
import concourse.mybir as mybir
import concourse.tile as tile
from concourse import bacc
from concourse.bass_utils import run_bass_kernel_spmd

N_CORES = 8
T = 2048
HID = 2048
NQ, NKV, D = 16, 8, 128
HQ = NQ // N_CORES  # q heads per core = 2
EPS = 1e-6
SCALE = D**-0.5
SHIFT = 2.0
WS = 64.0   # Wq/Wk pre-scale (cancels in RMS norm)
VS = 32.0   # Wv pre-scale == ctx scale alpha (fp8 range)
BETA = 64.0  # Wo pre-scale (fp8 range); host divides by VS*BETA

P = 128
H = D // 2
KP = HID // 256     # 8 K-pair chunks of 256
NTR = T // 512      # 4 T-ranges of 512

F32 = mybir.dt.float32
BF16 = mybir.dt.bfloat16
F8 = mybir.dt.float8e4
DR = mybir.MatmulPerfMode.DoubleRow
ACT_EXP = mybir.ActivationFunctionType.Exp
ACT_SQRT = mybir.ActivationFunctionType.Sqrt
ACT_SQUARE = mybir.ActivationFunctionType.Square
MIN = mybir.AluOpType.min
MULT = mybir.AluOpType.mult
SUB = mybir.AluOpType.subtract

QR_ORDER = [0, 1, 2, 3]  # last one takes the bf16-copy output path


def build_nc():
    nc = bacc.Bacc("TRN2", target_bir_lowering=False, debug=False,
                   num_devices=N_CORES)

    # ---- DRAM tensors (names = in_map keys) ----
    xh = nc.dram_tensor("xh", [P, KP, 2, T], F8, kind="ExternalInput")
    xl = nc.dram_tensor("xl", [P, KP, 2, T], F8, kind="ExternalInput")
    wqh = nc.dram_tensor("wqh", [P, KP, 2, HQ * D], F8, kind="ExternalInput")
    wql = nc.dram_tensor("wql", [P, KP, 2, HQ * D], F8, kind="ExternalInput")
    wkh = nc.dram_tensor("wkh", [P, KP, 2, D], F8, kind="ExternalInput")
    wkl = nc.dram_tensor("wkl", [P, KP, 2, D], F8, kind="ExternalInput")
    wvh = nc.dram_tensor("wvh", [P, KP, 2, D], F8, kind="ExternalInput")
    wvl = nc.dram_tensor("wvl", [P, KP, 2, D], F8, kind="ExternalInput")
    woh = nc.dram_tensor("woh", [P, HQ, HID], F8, kind="ExternalInput")
    wol = nc.dram_tensor("wol", [P, HQ, HID], F8, kind="ExternalInput")
    cosT = nc.dram_tensor("cosT", [P, T], BF16, kind="ExternalInput")
    sinT = nc.dram_tensor("sinT", [P, T], BF16, kind="ExternalInput")
    qw = nc.dram_tensor("qw", [P, 1], F32, kind="ExternalInput")
    kw = nc.dram_tensor("kw", [P, 1], F32, kind="ExternalInput")
    masks = nc.dram_tensor("masks", [P, P], BF16, kind="ExternalInput")
    out = nc.dram_tensor("out", [T, HID], BF16, kind="ExternalOutput")

    with tile.TileContext(nc) as tc:
        with (
            tc.tile_pool(name="cst", bufs=1) as cst,
            tc.tile_pool(name="fin", bufs=1) as fin,
        ):
            # ---------- constants / weights resident in SBUF ----------
            xh_sb = cst.tile([P, KP, 2, T], F8)
            xl_sb = cst.tile([P, KP, 2, T], F8)
            wqh_sb = cst.tile([P, KP, 2, HQ * D], F8)
            wql_sb = cst.tile([P, KP, 2, HQ * D], F8)
            wkh_sb = cst.tile([P, KP, 2, D], F8)
            wkl_sb = cst.tile([P, KP, 2, D], F8)
            wvh_sb = cst.tile([P, KP, 2, D], F8)
            wvl_sb = cst.tile([P, KP, 2, D], F8)
            woh_sb = cst.tile([P, HQ, HID], F8)
            wol_sb = cst.tile([P, HQ, HID], F8)
            masks_sb = cst.tile([P, P], BF16)
            cos_sb = cst.tile([P, T], BF16)
            sin_sb = cst.tile([P, T], BF16)
            qw_sb = cst.tile([P, 1], F32)
            kw_sb = cst.tile([P, 1], F32)
            nc.scalar.dma_start(qw_sb[:], qw[:])
            nc.scalar.dma_start(kw_sb[:], kw[:])
            ones_b = cst.tile([P, 1], BF16)
            nc.vector.memset(ones_b[:], 1.0)
            # DoubleRow ldweights requires the 2-plane dim step % 16 == 0
            w1_8 = cst.tile([P, 2, 16], F8)
            nc.vector.memset(w1_8[:], 1.0)
            shift_sb = cst.tile([P, 1], F32)
            nc.vector.memset(shift_sb[:], -SHIFT)

            # post RMS+RoPE q/k in bf16 (d on partitions)
            qT = [fin.tile([P, T], BF16, name=f"qT_{s}") for s in range(3)]
            # V (VS x): fp8 plane-pairs (plane = st parity) + bf16 st 0-3
            vp = fin.tile([P, T // 256, 2, D], F8)
            v0b = fin.tile([P, 4, D], BF16)
            # normalized ctx (VS x), fp8 hi/lo, plane = head
            ctxC = fin.tile([P, HQ, T], F8)
            ctxL = fin.tile([P, HQ, T], F8)

            # ==== SBUF pools span both phases (a reopened pool would reuse
            # phase A's region and serialize phase C behind its last reader)
            with (
                tc.tile_pool(name="tmpp", bufs=4) as tmpp,
                tc.tile_pool(name="atp", bufs=5) as atp,
                tc.tile_pool(name="adp", bufs=3) as adp,
                tc.tile_pool(name="cfp", bufs=2) as cfp,
                tc.tile_pool(name="otp", bufs=6) as otp,
                tc.tile_pool(name="attp", bufs=4) as attp,
            ):
              # ==== Phase A (split-fp8 DR projections) + B (RMS+RoPE) ====
              deferred = []
              with (
                tc.tile_pool(name="psA", bufs=4, space="PSUM") as psA,
                tc.tile_pool(name="psV", bufs=2, space="PSUM") as psV,
                tc.tile_pool(name="psB", bufs=2, space="PSUM") as psB,
              ):
                for tr in range(NTR):
                    ts = slice(tr * 512, (tr + 1) * 512)
                    if tr == 0:
                        # x is fully SBUF-resident. All phase-A-critical
                        # loads go through ONE queue (SP) in exact need
                        # order -- DMA_ENGINES serves transfers in DGE
                        # arrival order, so multiple queues let late bulk
                        # loads cut ahead of soon-needed weights. The tr0
                        # chunks are kp-split so the PE can start early.
                        nc.sync.dma_start(wqh_sb[:], wqh[:])
                        nc.sync.dma_start(xh_sb[:, 0:4, :, 0:512],
                                          xh[:, 0:4, :, 0:512])
                        nc.sync.dma_start(xh_sb[:, 4:8, :, 0:512],
                                          xh[:, 4:8, :, 0:512])
                        nc.sync.dma_start(wkh_sb[:], wkh[:])
                        nc.sync.dma_start(wvh_sb[:], wvh[:])
                        nc.sync.dma_start(wql_sb[:], wql[:])
                        nc.sync.dma_start(wkl_sb[:], wkl[:])
                        nc.sync.dma_start(wvl_sb[:], wvl[:])
                        nc.sync.dma_start(xl_sb[:, 0:4, :, 0:512],
                                          xl[:, 0:4, :, 0:512])
                        nc.sync.dma_start(xl_sb[:, 4:8, :, 0:512],
                                          xl[:, 4:8, :, 0:512])
                        for r in range(1, NTR):
                            rs = slice(r * 512, (r + 1) * 512)
                            nc.sync.dma_start(xh_sb[:, :, :, rs],
                                              xh[:, :, :, rs])
                            nc.sync.dma_start(xl_sb[:, :, :, rs],
                                              xl[:, :, :, rs])
                        # cos/sin at the tail of the SP stream: program
                        # order precedes tr0's RoPE reads (required for dep
                        # tracking) but the transfers queue after the x
                        # stream (RoPE tolerates late cos/sin)
                        nc.sync.dma_start(cos_sb[:], cosT[:])
                        nc.sync.dma_start(sin_sb[:], sinT[:])
                    if tr == 2:
                        nc.gpsimd.dma_start(masks_sb[:], masks[:])
                        nc.gpsimd.dma_start(woh_sb[:], woh[:])
                        nc.gpsimd.dma_start(wol_sb[:], wol[:])

                    terms = ((wqh_sb, wkh_sb, wvh_sb, xh_sb),
                             (wql_sb, wkl_sb, wvl_sb, xh_sb),
                             (wqh_sb, wkh_sb, wvh_sb, xl_sb))

                    # --- projections q0, q1, k + direct-transposed v ---
                    # tr 0 runs term-outer so the lo-weight / x-lo DMAs are
                    # needed as late as possible while the stream warms up;
                    # later trs run s-outer (one psA tile in flight each)
                    psv = psV.tile([P, 4, D], F32, name="psv")

                    def v_mm(wv_t, xt_, kp, j, first, last):
                        jts = slice(tr * 512 + j * P,
                                    tr * 512 + (j + 1) * P)
                        nc.tensor.matmul(
                            psv[:, j, :], xt_[:, kp, :, jts],
                            wv_t[:, kp, :, :],
                            perf_mode=DR, start=first, stop=last,
                        )

                    def qk_mm(ps, s, wq_t, wk_t, xt_, kp, first, last):
                        wt = wq_t if s < 2 else wk_t
                        cs = slice(s * D, (s + 1) * D) if s < 2 \
                            else slice(0, D)
                        nc.tensor.matmul(
                            ps[:], wt[:, kp, :, cs], xt_[:, kp, :, ts],
                            perf_mode=DR, start=first, stop=last,
                        )

                    if tr == 0:
                        raw = [psA.tile([P, 512], F32, name="psA_t")
                               for _ in range(3)]
                        for t_, (wq_t, wk_t, wv_t, xt_) in enumerate(terms):
                            for kp in range(KP):
                                first = t_ == 0 and kp == 0
                                last = t_ == 2 and kp == KP - 1
                                for s in range(3):
                                    qk_mm(raw[s], s, wq_t, wk_t, xt_,
                                          kp, first, last)
                    else:
                        raw = []
                        for s in range(3):
                            ps = psA.tile([P, 512], F32, name="psA_t")
                            i = 0
                            for wq_t, wk_t, _, xt_ in terms:
                                for kp in range(KP):
                                    qk_mm(ps, s, wq_t, wk_t, xt_, kp,
                                          i == 0, i == 3 * KP - 1)
                                    i += 1
                            raw.append(ps)
                    # one accumulation group for the whole bank: the
                    # first start lazily zeroes the full 2KB zero region
                    for j in range(4):
                        i = 0
                        for _, _, wv_t, xt_ in terms:
                            for kp in range(KP):
                                v_mm(wv_t, xt_, kp, j,
                                     j == 0 and i == 0,
                                     j == 3 and i == 3 * KP - 1)
                                i += 1
                    for j in range(4):
                        st = 4 * tr + j
                        nc.vector.tensor_copy(vp[:, st // 2, st % 2, :],
                                              psv[:, j, :])
                        if tr == 0:
                            nc.vector.tensor_copy(v0b[:, st, :], psv[:, j, :])

                    # --- B: RMS norm + RoPE for q0, q1, k (bf16) ---
                    # tr3's DVE/Pool back-half (bcast+stt+RoPE) is deferred
                    # past qr0's attention so phase C's DVE work isn't
                    # queued behind it (qr3 needs tr3's qT much later)
                    for s in range(3):
                        w_sb = qw_sb if s < 2 else kw_sb
                        # free the psA bank early for the next projection /
                        # the phase-C PSUM pools (Pool has slack here)
                        src = tmpp.tile([P, 512], F32, name="src")
                        nc.scalar.copy(src[:], raw[s][:])
                        sq = tmpp.tile([P, 512], BF16, name="sq")
                        nc.scalar.activation(sq[:], src[:], ACT_SQUARE)
                        ssum = psB.tile([1, 512], F32, name="ssum")
                        nc.tensor.matmul(ssum[:], ones_b[:], sq[:],
                                         start=True, stop=True)
                        # src holds 64*q; host folds sqrt(D) into q/k norm
                        # weights, eps is negligible vs ssum ~ 3e5
                        rstd = tmpp.tile([1, 512], F32, name="rstd")
                        nc.scalar.activation(rstd[:], ssum[:], ACT_SQRT)
                        rinv = tmpp.tile([1, 512], F32, name="rinv")
                        nc.vector.reciprocal_approx_fast(rinv[:], rstd[:])

                        def back_half(s=s, w_sb=w_sb, src=src, rinv=rinv,
                                      ts=ts):
                            rb = tmpp.tile([P, 512], F32, name="rb")
                            nc.gpsimd.partition_broadcast(rb[:], rinv[:])
                            nq = tmpp.tile([P, 512], BF16, name="nq")
                            nc.vector.scalar_tensor_tensor(
                                nq[:], src[:], w_sb[:], rb[:], MULT, MULT,
                            )
                            # RoPE: sin pre-rolled by 64 partitions with the
                            # rotate-half sign folded in; one full-width add
                            psn = tmpp.tile([P, 512], BF16, name="psn")
                            nc.vector.tensor_mul(psn[0:H, :], nq[H:D, :],
                                                 sin_sb[H:D, ts])
                            nc.vector.tensor_mul(psn[H:D, :], nq[0:H, :],
                                                 sin_sb[0:H, ts])
                            pc = tmpp.tile([P, 512], BF16, name="pc")
                            nc.vector.tensor_mul(pc[:], nq[:], cos_sb[:, ts])
                            nc.vector.tensor_add(qT[s][:, ts], pc[:], psn[:])

                        if tr == NTR - 1:
                            deferred.append(back_half)
                        else:
                            back_half()

              # ===== Phase C: attention + o_proj =====
              with (
                tc.tile_pool(name="psP", bufs=2, space="PSUM") as psP,
                tc.tile_pool(name="psCX", bufs=2, space="PSUM") as psCX,
                tc.tile_pool(name="psSM", bufs=1, space="PSUM") as psSM,
                tc.tile_pool(name="psD", bufs=3, space="PSUM") as psD,
              ):
                kT = qT[2]
                pending = []

                def make_task(qr, tt, nr, idx, pools=None):
                    abs_tt = 4 * qr + tt
                    tts = slice(abs_tt * P, (abs_tt + 1) * P)
                    ns = slice(nr * 512, (nr + 1) * 512)

                    def go():
                        if pools is None:
                            ps = psD.tile([P, 512], F32, name="psD_t")
                        else:
                            # tail: attention PSUM banks are free; borrow
                            # them so the last o_proj burst isn't throttled
                            # by psD recycling
                            pool, tag = pools[idx % len(pools)]
                            ps = pool.tile([P, 512], F32, name=tag)
                        for i, (cs, ws) in enumerate(
                                ((ctxC, woh_sb), (ctxL, woh_sb),
                                 (ctxC, wol_sb))):
                            nc.tensor.matmul(
                                ps[:], cs[:, :, tts], ws[:, :, ns],
                                perf_mode=DR,
                                start=(i == 0), stop=(i == 2))
                        ot = otp.tile([P, 512], BF16, name="ot")
                        # GPSIMD cannot read PSUM; during attention the
                        # copies go Act-heavy (DVE is loaded), in the tail
                        # burst they alternate evenly with the idle DVE
                        if pools is None:
                            dve = idx % 3 == 2
                        else:
                            dve = idx % 2 == 0
                        if dve:
                            nc.vector.tensor_copy(ot[:], ps[:])
                        else:
                            nc.scalar.copy(ot[:], ps[:])
                        # out DMAs only on sync: a dma_start blocks its
                        # issuing engine's SEQ until the copy dependency
                        # resolves, so compute queues must not carry them
                        nc.sync.dma_start(out[tts, ns], ot[:])
                    return go

                def emit_fill(k):
                    for _ in range(min(k, len(pending))):
                        pending.pop(0)()

                def attn_part(qr, h, qoff, qlen):
                    """Attention for queries [qr*512+qoff, +qlen) of head h."""
                    qs = slice(qr * 512 + qoff, qr * 512 + qoff + qlen)
                    at_dt = BF16 if qr == 0 else F8
                    n_off = 2 * qr
                    ctx_ps = psCX.tile([P, 512], F32, name="ctx_ps")
                    sums_t = psSM.tile([1, 512], F32, name="sums_t")
                    sums_ps = sums_t[:]
                    j0, j1 = qoff // P, (qoff + qlen) // P
                    # --- fully-causal pairs below the diagonal block ---
                    for pi in range(n_off):
                        at = atp.tile([P, 2, 512], at_dt, name="at")
                        for half in range(2):
                            st = 2 * pi + half
                            s_ps = psP.tile([P, 512], F32, name="s_t")
                            nc.tensor.matmul(
                                s_ps[:, 0:qlen], kT[:, st * P:(st + 1) * P],
                                qT[h][:, qs], start=True, stop=True)
                            nc.scalar.activation(
                                at[:, half, 0:qlen], s_ps[:, 0:qlen],
                                ACT_EXP, scale=SCALE, bias=shift_sb[:])
                        nc.tensor.matmul(
                            ctx_ps[:, 0:qlen], vp[:, pi, :, :],
                            at[:, :, 0:qlen],
                            perf_mode=DR, start=(pi == 0), stop=False)
                        nc.tensor.matmul(
                            sums_ps[:, 0:qlen], w1_8[:, :, 0:1],
                            at[:, :, 0:qlen],
                            perf_mode=DR, start=(pi == 0), stop=False)
                        emit_fill(1)
                    # --- diagonal block, 128-query granular ---
                    for j in range(j0, j1):
                        jr = j - j0
                        jsl = slice(jr * P, (jr + 1) * P)
                        qjs = slice(qr * 512 + j * P, qr * 512 + (j + 1) * P)
                        sd = psP.tile([P, 4, P], F32, name="s_t")
                        for i in range(j + 1):
                            st = 4 * qr + i
                            nc.tensor.matmul(
                                sd[:, i, :], kT[:, st * P:(st + 1) * P],
                                qT[h][:, qjs], start=True, stop=True)
                        ad = adp.tile([P, 4, P], at_dt, name="ad")
                        nc.scalar.activation(
                            ad[:, 0:j + 1, :], sd[:, 0:j + 1, :],
                            ACT_EXP, scale=SCALE, bias=shift_sb[:])
                        # only the true-diagonal tile needs masking
                        nc.vector.tensor_tensor(
                            ad[:, j, :], ad[:, j, :], masks_sb[:], MIN)
                        if qr == 0:
                            # one group: first start zeroes the whole bank
                            for i in range(j + 1):
                                st_ = j == j0 and i == 0
                                fin0 = j == j1 - 1 and i == j
                                nc.tensor.matmul(
                                    ctx_ps[:, jsl], v0b[:, i, :],
                                    ad[:, i, :], start=st_, stop=fin0)
                                nc.tensor.matmul(
                                    sums_ps[:, jsl], ones_b[:],
                                    ad[:, i, :], start=st_, stop=fin0)
                        else:
                            # one group (opened by off-diag pi==0 over the
                            # full query range): stop only on the final
                            # matmul of the last diag subtile
                            fin = j == j1 - 1
                            np_full = (j + 1) // 2
                            for p_ in range(np_full):
                                last_ = (j % 2 == 1) and (p_ == np_full - 1)
                                nc.tensor.matmul(
                                    ctx_ps[:, jsl],
                                    vp[:, 2 * qr + p_, :, :],
                                    ad[:, 2 * p_:2 * p_ + 2, :],
                                    perf_mode=DR, start=False,
                                    stop=(fin and last_))
                                nc.tensor.matmul(
                                    sums_ps[:, jsl], w1_8[:, :, 0:1],
                                    ad[:, 2 * p_:2 * p_ + 2, :],
                                    perf_mode=DR, start=False,
                                    stop=(fin and last_))
                            if j % 2 == 0:  # odd plane count: tail tile
                                nc.tensor.matmul(
                                    ctx_ps[:, jsl],
                                    vp[:, 2 * qr + j // 2, j % 2, :],
                                    ad[:, j, :], start=False, stop=fin)
                                nc.tensor.matmul(
                                    sums_ps[:, jsl], w1_8[:, 0, 0:1],
                                    ad[:, j, :], start=False, stop=fin)
                        emit_fill(1)
                    # --- normalize + fp8 hi/lo split of ctx ---
                    recip = attp.tile([1, 512], F32, name="recip")
                    nc.vector.reciprocal_approx_fast(
                        recip[:, 0:qlen], sums_ps[:, 0:qlen])
                    rb = attp.tile([P, 512], F32, name="rbc")
                    nc.gpsimd.partition_broadcast(
                        rb[:, 0:qlen], recip[:, 0:qlen])
                    cf = cfp.tile([P, 512], F32, name="cf")
                    nc.vector.tensor_mul(
                        cf[:, 0:qlen], ctx_ps[:, 0:qlen], rb[:, 0:qlen])
                    nc.gpsimd.tensor_copy(ctxC[:, h, qs], cf[:, 0:qlen])
                    nc.vector.scalar_tensor_tensor(
                        ctxL[:, h, qs], cf[:, 0:qlen], 1.0, ctxC[:, h, qs],
                        MULT, SUB)
                    emit_fill(1)

                for qi, qr in enumerate(QR_ORDER):
                    last = qi == len(QR_ORDER) - 1
                    # the last block runs as two query halves so half of
                    # its o_proj overlaps the second half's attention
                    parts = ((0, 256), (256, 256)) if last else ((0, 512),)
                    for qoff, qlen in parts:
                        for h in range(HQ):
                            attn_part(qr, h, qoff, qlen)
                        tailp = [(psD, "psD_t"), (psP, "s_t"),
                                 (psCX, "ctx_ps")] \
                            if last and qoff > 0 else None
                        for tt in range(qoff // P, (qoff + qlen) // P):
                            for nr in range(4):
                                pending.append(
                                    make_task(qr, tt, nr, 4 * tt + nr,
                                              pools=tailp))
                    if qi == 0:
                        for fn in deferred:
                            fn()
                        deferred.clear()
                    if last:
                        emit_fill(len(pending))

    nc.compile()
    return nc


_NC_CACHE = None


def get_nc():
    global _NC_CACHE
    if _NC_CACHE is None:
        _NC_CACHE = build_nc()
    return _NC_CACHE


F8NP = ml_dtypes.float8_e4m3
BF16NP = ml_dtypes.bfloat16


def _fold_hid(a):
    """[HID, C] -> [P, KP, 2, C] with hid = kp*256 + pl*128 + p."""
    c = a.shape[1]
    return np.ascontiguousarray(
        a.reshape(KP, 2, P, c).transpose(2, 0, 1, 3))


def _split8(a):
    hi = a.astype(F8NP)
    lo = (a - hi.astype(np.float32)).astype(F8NP)
    return hi, lo


def make_in_maps(x, cos, sin, Wq, Wk, Wv, Wo, q_norm_w, k_norm_w):
    x = np.asarray(x, dtype=np.float32).reshape(T, HID)
    xf = _fold_hid(np.ascontiguousarray(x.T).reshape(HID, T))
    xh, xl = _split8(xf)
    cosb = np.ascontiguousarray(
        np.asarray(cos, np.float32).T).astype(BF16NP)
    # rolled by 64 with rotate-half signs folded in:
    # psn[0:64] (subtracted in ref) uses rows 64:128 -> negate those rows
    sr = np.roll(np.asarray(sin, np.float32).T, 64, axis=0)
    sr[64:, :] *= -1.0
    sinb = np.ascontiguousarray(sr).astype(BF16NP)
    # sqrt(D) folded here: kernel computes rinv = (sum (64 q)^2)^-0.5
    sqd = np.float32(np.sqrt(D))
    qwa = np.ascontiguousarray(
        np.asarray(q_norm_w, np.float32).reshape(D, 1) * sqd)
    kwa = np.ascontiguousarray(
        np.asarray(k_norm_w, np.float32).reshape(D, 1) * sqd)
    si = np.arange(P)[:, None]
    qi = np.arange(P)[None, :]
    masks = np.where(si <= qi, 240.0, 0.0).astype(BF16NP)
    Wq = np.asarray(Wq, np.float32) * WS
    Wk = np.asarray(Wk, np.float32) * WS
    Wv = np.asarray(Wv, np.float32) * VS
    Wo = np.asarray(Wo, np.float32) * BETA
    in_maps = []
    for c in range(N_CORES):
        wqh_, wql_ = _split8(_fold_hid(Wq[:, c * HQ * D:(c + 1) * HQ * D]))
        wkh_, wkl_ = _split8(_fold_hid(Wk[:, c * D:(c + 1) * D]))
        wvh_, wvl_ = _split8(_fold_hid(Wv[:, c * D:(c + 1) * D]))
        wo_ = np.ascontiguousarray(
            Wo[c * HQ * D:(c + 1) * HQ * D, :].reshape(HQ, P, HID)
            .transpose(1, 0, 2))
        woh_, wol_ = _split8(wo_)
        in_maps.append({
            "xh": xh, "xl": xl,
            "wqh": wqh_, "wql": wql_,
            "wkh": wkh_, "wkl": wkl_,
            "wvh": wvh_, "wvl": wvl_,
            "woh": woh_, "wol": wol_,
            "cosT": cosb, "sinT": sinb,
            "qw": qwa, "kw": kwa,
            "masks": masks,
        })
    return in_maps


def kernel(x, cos, sin, Wq, Wk, Wv, Wo, q_norm_w, k_norm_w):
    nc = get_nc()
    in_maps = make_in_maps(x, cos, sin, Wq, Wk, Wv, Wo, q_norm_w, k_norm_w)
    res = run_bass_kernel_spmd(nc, in_maps, core_ids=list(range(N_CORES)))
    acc = np.zeros((T, HID), dtype=np.float32)
    for c in range(N_CORES):
        acc += np.asarray(res.results[c]["out"], np.float32)
    acc *= 1.0 / (VS * BETA)
    return acc.reshape(1, T, HID)


# revision 69
# speedup vs baseline: 1.1892x; 1.1257x over previous
"""GQA attention block (B=1, T=2048, HID=2048, NQ=16, NKV=8, D=128) on 8 TRN2
NeuronCores.

Sharding: tensor-parallel over heads. Core c owns q-heads {2c, 2c+1} and
kv-head c. The 8 partial outputs are summed on the host (scaled 1/(VS*BETA)).

v2 speed strategy (tuned against the TimelineSim cost model, validated on
device + interpreter + f64 reference):
  - projections: 3-term split-fp8 (xh*wh + xl*wh + xh*wl) with K=256
    DoubleRow matmuls (0.5 cyc/row in the cost model). Wq/Wk pre-scaled by
    WS=64 (cancels through RMS norm), Wv by VS=32. x is SBUF-resident,
    streamed on one queue in exact need order (DMA_ENGINES serves in DGE
    arrival order). tr0 runs term-outer so lo-weight/x-lo land later.
  - V is projected directly transposed ([t, d] tiles, stationary = x
    chunk), so no PE transposes are needed.
  - q/k: RMS-norm (Act sq/sqrt + DVE recip; sqrt(D) folded into the host
    norm weights, eps negligible) + RoPE on DVE in bf16. tr3's DVE
    back-half is deferred past qr0's attention (qr3 needs it much later).
  - attention: at = exp(score/sqrt(D) - 2), fp8 for q-rows >= 512, bf16
    below. The diagonal 512x512 block is 128-query granular: only needed
    key tiles are computed and only the true-diagonal tile is min-masked
    (mask in {0, 240}: min(sat, 0) = 0 kills acausal fp8-overflowed exp).
    One PSUM accumulation group per (block, head): single start lazily
    zeroes the 2KB region, single stop at the end.
  - denominators: ones-stationary matmuls accumulated alongside ctx.
  - o_proj: 3-term split-fp8 DoubleRow with BOTH heads packed into K=256;
    ctx is normalized then split hi/lo (DVE mul + Pool fp8 copy + DVE
    subtract); Wo pre-scaled by BETA=64, ctx carries VS=32, host divides
    by 2048. PSUM -> bf16 out copies run on DVE (Act is exp-bound),
    alternating with Act only in the tail burst; out DMAs issue on sync
    only (a dma_start blocks its queue's SEQ until the copy resolves).
  - schedule: q-blocks in order 0..3; each block's 16 o_proj tiles are
    queued and interleaved as PE filler into the NEXT block's attention
    (concentrated at diag/normalize points where Act latency would
    otherwise stall the PE). The last block runs as two 256-query halves
    (half the tail overlaps the second half's attention), and its tiles
    borrow the freed attention PSUM banks.
"""

import sys

sys.path.insert(0, "/opt/trn_rl_repo")

import numpy as np
import ml_dtypes

import concourse.bass as bass  # noqa: F401  (bass must import before tile)
import concourse.mybir as mybir
import concourse.tile as tile
from concourse import bacc
from concourse.bass_utils import run_bass_kernel_spmd

N_CORES = 8
T = 2048
HID = 2048
NQ, NKV, D = 16, 8, 128
HQ = NQ // N_CORES  # q heads per core = 2
EPS = 1e-6
SCALE = D**-0.5
SHIFT = 2.0
WS = 64.0   # Wq/Wk pre-scale (cancels in RMS norm)
VS = 32.0   # Wv pre-scale == ctx scale alpha (fp8 range)
BETA = 64.0  # Wo pre-scale (fp8 range); host divides by VS*BETA

P = 128
H = D // 2
KP = HID // 256     # 8 K-pair chunks of 256
NTR = T // 512      # 4 T-ranges of 512

F32 = mybir.dt.float32
BF16 = mybir.dt.bfloat16
F8 = mybir.dt.float8e4
DR = mybir.MatmulPerfMode.DoubleRow
ACT_EXP = mybir.ActivationFunctionType.Exp
ACT_SQRT = mybir.ActivationFunctionType.Sqrt
ACT_SQUARE = mybir.ActivationFunctionType.Square
MIN = mybir.AluOpType.min
MULT = mybir.AluOpType.mult
SUB = mybir.AluOpType.subtract

QR_ORDER = [0, 1, 2, 3]  # last one takes the bf16-copy output path


def build_nc():
    nc = bacc.Bacc("TRN2", target_bir_lowering=False, debug=False,
                   num_devices=N_CORES)

    # ---- DRAM tensors (names = in_map keys) ----
    xh = nc.dram_tensor("xh", [P, KP, 2, T], F8, kind="ExternalInput")
    xl = nc.dram_tensor("xl", [P, KP, 2, T], F8, kind="ExternalInput")
    wqh = nc.dram_tensor("wqh", [P, KP, 2, HQ * D], F8, kind="ExternalInput")
    wql = nc.dram_tensor("wql", [P, KP, 2, HQ * D], F8, kind="ExternalInput")
    wkh = nc.dram_tensor("wkh", [P, KP, 2, D], F8, kind="ExternalInput")
    wkl = nc.dram_tensor("wkl", [P, KP, 2, D], F8, kind="ExternalInput")
    wvh = nc.dram_tensor("wvh", [P, KP, 2, D], F8, kind="ExternalInput")
    wvl = nc.dram_tensor("wvl", [P, KP, 2, D], F8, kind="ExternalInput")
    woh = nc.dram_tensor("woh", [P, HQ, HID], F8, kind="ExternalInput")
    wol = nc.dram_tensor("wol", [P, HQ, HID], F8, kind="ExternalInput")
    cosT = nc.dram_tensor("cosT", [P, T], BF16, kind="ExternalInput")
    sinT = nc.dram_tensor("sinT", [P, T], BF16, kind="ExternalInput")
    qw = nc.dram_tensor("qw", [P, 1], F32, kind="ExternalInput")
    kw = nc.dram_tensor("kw", [P, 1], F32, kind="ExternalInput")
    masks = nc.dram_tensor("masks", [P, P], BF16, kind="ExternalInput")
    out = nc.dram_tensor("out", [T, HID], BF16, kind="ExternalOutput")

    with tile.TileContext(nc) as tc:
        with (
            tc.tile_pool(name="cst", bufs=1) as cst,
            tc.tile_pool(name="fin", bufs=1) as fin,
        ):
            # ---------- constants / weights resident in SBUF ----------
            xh_sb = cst.tile([P, KP, 2, T], F8)
            xl_sb = cst.tile([P, KP, 2, T], F8)
            wqh_sb = cst.tile([P, KP, 2, HQ * D], F8)
            wql_sb = cst.tile([P, KP, 2, HQ * D], F8)
            wkh_sb = cst.tile([P, KP, 2, D], F8)
            wkl_sb = cst.tile([P, KP, 2, D], F8)
            wvh_sb = cst.tile([P, KP, 2, D], F8)
            wvl_sb = cst.tile([P, KP, 2, D], F8)
            woh_sb = cst.tile([P, HQ, HID], F8)
            wol_sb = cst.tile([P, HQ, HID], F8)
            masks_sb = cst.tile([P, P], BF16)
            cos_sb = cst.tile([P, T], BF16)
            sin_sb = cst.tile([P, T], BF16)
            qw_sb = cst.tile([P, 1], F32)
            kw_sb = cst.tile([P, 1], F32)
            nc.scalar.dma_start(qw_sb[:], qw[:])
            nc.scalar.dma_start(kw_sb[:], kw[:])
            ones_b = cst.tile([P, 1], BF16)
            nc.vector.memset(ones_b[:], 1.0)
            # DoubleRow ldweights requires the 2-plane dim step % 16 == 0
            w1_8 = cst.tile([P, 2, 16], F8)
            nc.vector.memset(w1_8[:], 1.0)
            shift_sb = cst.tile([P, 1], F32)
            nc.vector.memset(shift_sb[:], -SHIFT)

            # post RMS+RoPE q/k in bf16 (d on partitions)
            qT = [fin.tile([P, T], BF16, name=f"qT_{s}") for s in range(3)]
            # V (VS x): fp8 plane-pairs (plane = st parity) + bf16 st 0-3
            vp = fin.tile([P, T // 256, 2, D], F8)
            v0b = fin.tile([P, 4, D], BF16)
            # normalized ctx (VS x), fp8 hi/lo, plane = head
            ctxC = fin.tile([P, HQ, T], F8)
            ctxL = fin.tile([P, HQ, T], F8)

            # ==== SBUF pools span both phases (a reopened pool would reuse
            # phase A's region and serialize phase C behind its last reader)
            with (
                tc.tile_pool(name="tmpp", bufs=4) as tmpp,
                tc.tile_pool(name="atp", bufs=6) as atp,
                tc.tile_pool(name="adp", bufs=4) as adp,
                tc.tile_pool(name="cfp", bufs=2) as cfp,
                tc.tile_pool(name="otp", bufs=6) as otp,
                tc.tile_pool(name="attp", bufs=4) as attp,
            ):
              # ==== Phase A (split-fp8 DR projections) + B (RMS+RoPE) ====
              deferred = []
              with (
                tc.tile_pool(name="psA", bufs=4, space="PSUM") as psA,
                tc.tile_pool(name="psV", bufs=2, space="PSUM") as psV,
                tc.tile_pool(name="psB", bufs=2, space="PSUM") as psB,
              ):
                for tr in range(NTR):
                    ts = slice(tr * 512, (tr + 1) * 512)
                    if tr == 0:
                        # x is fully SBUF-resident. All phase-A-critical
                        # loads go through ONE queue (SP) in exact need
                        # order -- DMA_ENGINES serves transfers in DGE
                        # arrival order, so multiple queues let late bulk
                        # loads cut ahead of soon-needed weights. The tr0
                        # chunks are kp-split so the PE can start early.
                        nc.sync.dma_start(wqh_sb[:], wqh[:])
                        nc.sync.dma_start(xh_sb[:, 0:4, :, 0:512],
                                          xh[:, 0:4, :, 0:512])
                        nc.sync.dma_start(xh_sb[:, 4:8, :, 0:512],
                                          xh[:, 4:8, :, 0:512])
                        nc.sync.dma_start(wkh_sb[:], wkh[:])
                        nc.sync.dma_start(wvh_sb[:], wvh[:])
                        nc.sync.dma_start(wql_sb[:], wql[:])
                        nc.sync.dma_start(wkl_sb[:], wkl[:])
                        nc.sync.dma_start(wvl_sb[:], wvl[:])
                        nc.sync.dma_start(xl_sb[:, 0:4, :, 0:512],
                                          xl[:, 0:4, :, 0:512])
                        nc.sync.dma_start(xl_sb[:, 4:8, :, 0:512],
                                          xl[:, 4:8, :, 0:512])
                        for r in range(1, NTR):
                            rs = slice(r * 512, (r + 1) * 512)
                            nc.sync.dma_start(xh_sb[:, :, :, rs],
                                              xh[:, :, :, rs])
                            nc.sync.dma_start(xl_sb[:, :, :, rs],
                                              xl[:, :, :, rs])
                        # cos/sin at the tail of the SP stream: program
                        # order precedes tr0's RoPE reads (required for dep
                        # tracking) but the transfers queue after the x
                        # stream (RoPE tolerates late cos/sin)
                        nc.sync.dma_start(cos_sb[:], cosT[:])
                        nc.sync.dma_start(sin_sb[:], sinT[:])
                    if tr == 2:
                        nc.gpsimd.dma_start(masks_sb[:], masks[:])
                        nc.gpsimd.dma_start(woh_sb[:], woh[:])
                        nc.gpsimd.dma_start(wol_sb[:], wol[:])

                    terms = ((wqh_sb, wkh_sb, wvh_sb, xh_sb),
                             (wql_sb, wkl_sb, wvl_sb, xh_sb),
                             (wqh_sb, wkh_sb, wvh_sb, xl_sb))

                    # --- projections q0, q1, k + direct-transposed v ---
                    # tr 0 runs term-outer so the lo-weight / x-lo DMAs are
                    # needed as late as possible while the stream warms up;
                    # later trs run s-outer (one psA tile in flight each)
                    psv = psV.tile([P, 4, D], F32, name="psv")

                    def v_mm(wv_t, xt_, kp, j, first, last):
                        jts = slice(tr * 512 + j * P,
                                    tr * 512 + (j + 1) * P)
                        nc.tensor.matmul(
                            psv[:, j, :], xt_[:, kp, :, jts],
                            wv_t[:, kp, :, :],
                            perf_mode=DR, start=first, stop=last,
                        )

                    def qk_mm(ps, s, wq_t, wk_t, xt_, kp, first, last):
                        wt = wq_t if s < 2 else wk_t
                        cs = slice(s * D, (s + 1) * D) if s < 2 \
                            else slice(0, D)
                        nc.tensor.matmul(
                            ps[:], wt[:, kp, :, cs], xt_[:, kp, :, ts],
                            perf_mode=DR, start=first, stop=last,
                        )

                    if tr == 0:
                        raw = [psA.tile([P, 512], F32, name="psA_t")
                               for _ in range(3)]
                        for t_, (wq_t, wk_t, wv_t, xt_) in enumerate(terms):
                            for kp in range(KP):
                                first = t_ == 0 and kp == 0
                                last = t_ == 2 and kp == KP - 1
                                for s in range(3):
                                    qk_mm(raw[s], s, wq_t, wk_t, xt_,
                                          kp, first, last)
                    else:
                        raw = []
                        for s in range(3):
                            ps = psA.tile([P, 512], F32, name="psA_t")
                            i = 0
                            for wq_t, wk_t, _, xt_ in terms:
                                for kp in range(KP):
                                    qk_mm(ps, s, wq_t, wk_t, xt_, kp,
                                          i == 0, i == 3 * KP - 1)
                                    i += 1
                            raw.append(ps)
                    # one accumulation group for the whole bank: the
                    # first start lazily zeroes the full 2KB zero region
                    for j in range(4):
                        i = 0
                        for _, _, wv_t, xt_ in terms:
                            for kp in range(KP):
                                v_mm(wv_t, xt_, kp, j,
                                     j == 0 and i == 0,
                                     j == 3 and i == 3 * KP - 1)
                                i += 1
                    for j in range(4):
                        st = 4 * tr + j
                        nc.vector.tensor_copy(vp[:, st // 2, st % 2, :],
                                              psv[:, j, :])
                        if tr == 0:
                            nc.vector.tensor_copy(v0b[:, st, :], psv[:, j, :])

                    # --- B: RMS norm + RoPE for q0, q1, k (bf16) ---
                    # tr3's DVE/Pool back-half (bcast+stt+RoPE) is deferred
                    # past qr0's attention so phase C's DVE work isn't
                    # queued behind it (qr3 needs tr3's qT much later)
                    for s in range(3):
                        w_sb = qw_sb if s < 2 else kw_sb
                        # free the psA bank early for the next projection /
                        # the phase-C PSUM pools (Pool has slack here)
                        src = tmpp.tile([P, 512], F32, name="src")
                        nc.scalar.copy(src[:], raw[s][:])
                        sq = tmpp.tile([P, 512], BF16, name="sq")
                        nc.scalar.activation(sq[:], src[:], ACT_SQUARE)
                        ssum = psB.tile([1, 512], F32, name="ssum")
                        nc.tensor.matmul(ssum[:], ones_b[:], sq[:],
                                         start=True, stop=True)
                        # src holds 64*q; host folds sqrt(D) into q/k norm
                        # weights, eps is negligible vs ssum ~ 3e5
                        rstd = tmpp.tile([1, 512], F32, name="rstd")
                        nc.scalar.activation(rstd[:], ssum[:], ACT_SQRT)
                        rinv = tmpp.tile([1, 512], F32, name="rinv")
                        nc.vector.reciprocal_approx_fast(rinv[:], rstd[:])

                        def back_half(s=s, w_sb=w_sb, src=src, rinv=rinv,
                                      ts=ts):
                            rb = tmpp.tile([P, 512], F32, name="rb")
                            nc.gpsimd.partition_broadcast(rb[:], rinv[:])
                            nq = tmpp.tile([P, 512], BF16, name="nq")
                            nc.vector.scalar_tensor_tensor(
                                nq[:], src[:], w_sb[:], rb[:], MULT, MULT,
                            )
                            # RoPE: sin pre-rolled by 64 partitions with the
                            # rotate-half sign folded in; one full-width add
                            psn = tmpp.tile([P, 512], BF16, name="psn")
                            nc.vector.tensor_mul(psn[0:H, :], nq[H:D, :],
                                                 sin_sb[H:D, ts])
                            nc.vector.tensor_mul(psn[H:D, :], nq[0:H, :],
                                                 sin_sb[0:H, ts])
                            pc = tmpp.tile([P, 512], BF16, name="pc")
                            nc.vector.tensor_mul(pc[:], nq[:], cos_sb[:, ts])
                            nc.vector.tensor_add(qT[s][:, ts], pc[:], psn[:])

                        if tr == NTR - 1:
                            deferred.append(back_half)
                        else:
                            back_half()

              # ===== Phase C: attention + o_proj =====
              with (
                tc.tile_pool(name="psP", bufs=2, space="PSUM") as psP,
                tc.tile_pool(name="psCX", bufs=2, space="PSUM") as psCX,
                tc.tile_pool(name="psSM", bufs=1, space="PSUM") as psSM,
                tc.tile_pool(name="psD", bufs=3, space="PSUM") as psD,
              ):
                kT = qT[2]
                pending = []

                def make_task(qr, tt, nr, idx, pools=None, tail=False):
                    abs_tt = 4 * qr + tt
                    tts = slice(abs_tt * P, (abs_tt + 1) * P)
                    ns = slice(nr * 512, (nr + 1) * 512)

                    def go():
                        if pools is None:
                            ps = psD.tile([P, 512], F32, name="psD_t")
                        else:
                            # tail: attention PSUM banks are free; borrow
                            # them so the last o_proj burst isn't throttled
                            # by psD recycling
                            pool, tag = pools[idx % len(pools)]
                            ps = pool.tile([P, 512], F32, name=tag)
                        for i, (cs, ws) in enumerate(
                                ((ctxC, woh_sb), (ctxL, woh_sb),
                                 (ctxC, wol_sb))):
                            nc.tensor.matmul(
                                ps[:], cs[:, :, tts], ws[:, :, ns],
                                perf_mode=DR,
                                start=(i == 0), stop=(i == 2))
                        ot = otp.tile([P, 512], BF16, name="ot")
                        # GPSIMD cannot read PSUM; during attention the
                        # copies go Act-heavy (DVE is loaded), in the tail
                        # burst they alternate evenly with the idle DVE
                        if tail and idx % 2 == 0:
                            nc.scalar.copy(ot[:], ps[:])
                        else:
                            nc.vector.tensor_copy(ot[:], ps[:])
                        # out DMAs only on sync: a dma_start blocks its
                        # issuing engine's SEQ until the copy dependency
                        # resolves, so compute queues must not carry them
                        nc.sync.dma_start(out[tts, ns], ot[:])
                    return go

                def emit_fill(k):
                    for _ in range(min(k, len(pending))):
                        pending.pop(0)()

                def attn_part(qr, h, qoff, qlen):
                    """Attention for queries [qr*512+qoff, +qlen) of head h."""
                    qs = slice(qr * 512 + qoff, qr * 512 + qoff + qlen)
                    at_dt = BF16 if qr == 0 else F8
                    n_off = 2 * qr
                    ctx_ps = psCX.tile([P, 512], F32, name="ctx_ps")
                    sums_t = psSM.tile([1, 512], F32, name="sums_t")
                    sums_ps = sums_t[:]
                    j0, j1 = qoff // P, (qoff + qlen) // P
                    # --- fully-causal pairs below the diagonal block ---
                    for pi in range(n_off):
                        at = atp.tile([P, 2, 512], at_dt, name="at")
                        for half in range(2):
                            st = 2 * pi + half
                            s_ps = psP.tile([P, 512], F32, name="s_t")
                            nc.tensor.matmul(
                                s_ps[:, 0:qlen], kT[:, st * P:(st + 1) * P],
                                qT[h][:, qs], start=True, stop=True)
                            nc.scalar.activation(
                                at[:, half, 0:qlen], s_ps[:, 0:qlen],
                                ACT_EXP, scale=SCALE, bias=shift_sb[:])
                        nc.tensor.matmul(
                            ctx_ps[:, 0:qlen], vp[:, pi, :, :],
                            at[:, :, 0:qlen],
                            perf_mode=DR, start=(pi == 0), stop=False)
                        nc.tensor.matmul(
                            sums_ps[:, 0:qlen], w1_8[:, :, 0:1],
                            at[:, :, 0:qlen],
                            perf_mode=DR, start=(pi == 0), stop=False)
                    # --- diagonal block, 128-query granular ---
                    for j in range(j0, j1):
                        jr = j - j0
                        jsl = slice(jr * P, (jr + 1) * P)
                        qjs = slice(qr * 512 + j * P, qr * 512 + (j + 1) * P)
                        sd = psP.tile([P, 4, P], F32, name="s_t")
                        for i in range(j + 1):
                            st = 4 * qr + i
                            nc.tensor.matmul(
                                sd[:, i, :], kT[:, st * P:(st + 1) * P],
                                qT[h][:, qjs], start=True, stop=True)
                        ad = adp.tile([P, 4, P], at_dt, name="ad")
                        nc.scalar.activation(
                            ad[:, 0:j + 1, :], sd[:, 0:j + 1, :],
                            ACT_EXP, scale=SCALE, bias=shift_sb[:])
                        # only the true-diagonal tile needs masking
                        nc.vector.tensor_tensor(
                            ad[:, j, :], ad[:, j, :], masks_sb[:], MIN)
                        if qr == 0:
                            # one group: first start zeroes the whole bank
                            for i in range(j + 1):
                                st_ = j == j0 and i == 0
                                fin0 = j == j1 - 1 and i == j
                                nc.tensor.matmul(
                                    ctx_ps[:, jsl], v0b[:, i, :],
                                    ad[:, i, :], start=st_, stop=fin0)
                                nc.tensor.matmul(
                                    sums_ps[:, jsl], ones_b[:],
                                    ad[:, i, :], start=st_, stop=fin0)
                        else:
                            # one group (opened by off-diag pi==0 over the
                            # full query range): stop only on the final
                            # matmul of the last diag subtile
                            fin = j == j1 - 1
                            np_full = (j + 1) // 2
                            for p_ in range(np_full):
                                last_ = (j % 2 == 1) and (p_ == np_full - 1)
                                nc.tensor.matmul(
                                    ctx_ps[:, jsl],
                                    vp[:, 2 * qr + p_, :, :],
                                    ad[:, 2 * p_:2 * p_ + 2, :],
                                    perf_mode=DR, start=False,
                                    stop=(fin and last_))
                                nc.tensor.matmul(
                                    sums_ps[:, jsl], w1_8[:, :, 0:1],
                                    ad[:, 2 * p_:2 * p_ + 2, :],
                                    perf_mode=DR, start=False,
                                    stop=(fin and last_))
                            if j % 2 == 0:  # odd plane count: tail tile
                                nc.tensor.matmul(
                                    ctx_ps[:, jsl],
                                    vp[:, 2 * qr + j // 2, j % 2, :],
                                    ad[:, j, :], start=False, stop=fin)
                                nc.tensor.matmul(
                                    sums_ps[:, jsl], w1_8[:, 0, 0:1],
                                    ad[:, j, :], start=False, stop=fin)
                        emit_fill(1)
                    # --- normalize + fp8 hi/lo split of ctx ---
                    recip = attp.tile([1, 512], F32, name="recip")
                    nc.vector.reciprocal_approx_fast(
                        recip[:, 0:qlen], sums_ps[:, 0:qlen])
                    rb = attp.tile([P, 512], F32, name="rbc")
                    nc.gpsimd.partition_broadcast(
                        rb[:, 0:qlen], recip[:, 0:qlen])
                    cf = cfp.tile([P, 512], F32, name="cf")
                    nc.vector.tensor_mul(
                        cf[:, 0:qlen], ctx_ps[:, 0:qlen], rb[:, 0:qlen])
                    nc.gpsimd.tensor_copy(ctxC[:, h, qs], cf[:, 0:qlen])
                    nc.vector.scalar_tensor_tensor(
                        ctxL[:, h, qs], cf[:, 0:qlen], 1.0, ctxC[:, h, qs],
                        MULT, SUB)
                    emit_fill(3)

                for qi, qr in enumerate(QR_ORDER):
                    last = qi == len(QR_ORDER) - 1
                    # the last block runs as two query halves so half of
                    # its o_proj overlaps the second half's attention
                    parts = ((0, 256), (256, 256)) if last else ((0, 512),)
                    for qoff, qlen in parts:
                        for h in range(HQ):
                            attn_part(qr, h, qoff, qlen)
                        tailp = [(psD, "psD_t"), (psP, "s_t"),
                                 (psCX, "ctx_ps")] \
                            if last and qoff > 0 else None
                        for tt in range(qoff // P, (qoff + qlen) // P):
                            for nr in range(4):
                                pending.append(
                                    make_task(qr, tt, nr, 4 * tt + nr,
                                              pools=tailp, tail=last))
                    if qi == 0:
                        for fn in deferred:
                            fn()
                        deferred.clear()
                    if last:
                        emit_fill(len(pending))

    nc.compile()
    return nc


_NC_CACHE = None


def get_nc():
    global _NC_CACHE
    if _NC_CACHE is None:
        _NC_CACHE = build_nc()
    return _NC_CACHE


F8NP = ml_dtypes.float8_e4m3
BF16NP = ml_dtypes.bfloat16


def _fold_hid(a):
    """[HID, C] -> [P, KP, 2, C] with hid = kp*256 + pl*128 + p."""
    c = a.shape[1]
    return np.ascontiguousarray(
        a.reshape(KP, 2, P, c).transpose(2, 0, 1, 3))


def _split8(a):
    hi = a.astype(F8NP)
    lo = (a - hi.astype(np.float32)).astype(F8NP)
    return hi, lo


def make_in_maps(x, cos, sin, Wq, Wk, Wv, Wo, q_norm_w, k_norm_w):
    x = np.asarray(x, dtype=np.float32).reshape(T, HID)
    xf = _fold_hid(np.ascontiguousarray(x.T).reshape(HID, T))
    xh, xl = _split8(xf)
    cosb = np.ascontiguousarray(
        np.asarray(cos, np.float32).T).astype(BF16NP)
    # rolled by 64 with rotate-half signs folded in:
    # psn[0:64] (subtracted in ref) uses rows 64:128 -> negate those rows
    sr = np.roll(np.asarray(sin, np.float32).T, 64, axis=0)
    sr[64:, :] *= -1.0
    sinb = np.ascontiguousarray(sr).astype(BF16NP)
    # sqrt(D) folded here: kernel computes rinv = (sum (64 q)^2)^-0.5
    sqd = np.float32(np.sqrt(D))
    qwa = np.ascontiguousarray(
        np.asarray(q_norm_w, np.float32).reshape(D, 1) * sqd)
    kwa = np.ascontiguousarray(
        np.asarray(k_norm_w, np.float32).reshape(D, 1) * sqd)
    si = np.arange(P)[:, None]
    qi = np.arange(P)[None, :]
    masks = np.where(si <= qi, 240.0, 0.0).astype(BF16NP)
    Wq = np.asarray(Wq, np.float32) * WS
    Wk = np.asarray(Wk, np.float32) * WS
    Wv = np.asarray(Wv, np.float32) * VS
    Wo = np.asarray(Wo, np.float32) * BETA
    in_maps = []
    for c in range(N_CORES):
        wqh_, wql_ = _split8(_fold_hid(Wq[:, c * HQ * D:(c + 1) * HQ * D]))
        wkh_, wkl_ = _split8(_fold_hid(Wk[:, c * D:(c + 1) * D]))
        wvh_, wvl_ = _split8(_fold_hid(Wv[:, c * D:(c + 1) * D]))
        wo_ = np.ascontiguousarray(
            Wo[c * HQ * D:(c + 1) * HQ * D, :].reshape(HQ, P, HID)
            .transpose(1, 0, 2))
        woh_, wol_ = _split8(wo_)
        in_maps.append({
            "xh": xh, "xl": xl,
            "wqh": wqh_, "wql": wql_,
            "wkh": wkh_, "wkl": wkl_,
            "wvh": wvh_, "wvl": wvl_,
            "woh": woh_, "wol": wol_,
            "cosT": cosb, "sinT": sinb,
            "qw": qwa, "kw": kwa,
            "masks": masks,
        })
    return in_maps


def kernel(x, cos, sin, Wq, Wk, Wv, Wo, q_norm_w, k_norm_w):
    nc = get_nc()
    in_maps = make_in_maps(x, cos, sin, Wq, Wk, Wv, Wo, q_norm_w, k_norm_w)
    res = run_bass_kernel_spmd(nc, in_maps, core_ids=list(range(N_CORES)))
    acc = np.zeros((T, HID), dtype=np.float32)
    for c in range(N_CORES):
        acc += np.asarray(res.results[c]["out"], np.float32)
    acc *= 1.0 / (VS * BETA)
    return acc.reshape(1, T, HID)


# revision 81
# speedup vs baseline: 1.1937x; 1.0038x over previous
"""GQA attention block (B=1, T=2048, HID=2048, NQ=16, NKV=8, D=128) on 8 TRN2
NeuronCores.

Sharding: tensor-parallel over heads. Core c owns q-heads {2c, 2c+1} and
kv-head c. The 8 partial outputs are summed on the host (scaled 1/(VS*BETA)).

v2 speed strategy (tuned against the TimelineSim cost model, validated on
device + interpreter + f64 reference):
  - projections: 3-term split-fp8 (xh*wh + xl*wh + xh*wl) with K=256
    DoubleRow matmuls (0.5 cyc/row in the cost model). Wq/Wk pre-scaled by
    WS=64 (cancels through RMS norm), Wv by VS=32. x is SBUF-resident,
    streamed on one queue in exact need order (DMA_ENGINES serves in DGE
    arrival order). tr0 runs term-outer so lo-weight/x-lo land later.
  - V is projected directly transposed ([t, d] tiles, stationary = x
    chunk), so no PE transposes are needed.
  - q/k: RMS-norm (Act sq/sqrt + DVE recip; sqrt(D) folded into the host
    norm weights, eps negligible) + RoPE on DVE in bf16. tr3's DVE
    back-half is deferred past qr0's attention (qr3 needs it much later).
  - attention: at = exp(score/sqrt(D) - 2), fp8 for q-rows >= 512, bf16
    below. The diagonal 512x512 block is 128-query granular: only needed
    key tiles are computed and only the true-diagonal tile is min-masked
    (mask in {0, 240}: min(sat, 0) = 0 kills acausal fp8-overflowed exp).
    One PSUM accumulation group per (block, head): single start lazily
    zeroes the 2KB region, single stop at the end.
  - denominators: ones-stationary matmuls accumulated alongside ctx.
  - o_proj: 3-term split-fp8 DoubleRow with BOTH heads packed into K=256;
    ctx is normalized then split hi/lo (DVE mul + Pool fp8 copy + DVE
    subtract); Wo pre-scaled by BETA=64, ctx carries VS=32, host divides
    by 2048. PSUM -> bf16 out copies run on DVE (Act is exp-bound),
    alternating with Act only in the tail burst; out DMAs issue on sync
    only (a dma_start blocks its queue's SEQ until the copy resolves).
  - schedule: q-blocks in order 0..3; each block's 16 o_proj tiles are
    queued and interleaved as PE filler into the NEXT block's attention
    (concentrated at diag/normalize points where Act latency would
    otherwise stall the PE). The last block runs as two 256-query halves
    (half the tail overlaps the second half's attention), and its tiles
    borrow the freed attention PSUM banks.
"""

import sys

sys.path.insert(0, "/opt/trn_rl_repo")

import numpy as np
import ml_dtypes

import concourse.bass as bass  # noqa: F401  (bass must import before tile)
import concourse.mybir as mybir
import concourse.tile as tile
from concourse import bacc
from concourse.bass_utils import run_bass_kernel_spmd

N_CORES = 8
T = 2048
HID = 2048
NQ, NKV, D = 16, 8, 128
HQ = NQ // N_CORES  # q heads per core = 2
EPS = 1e-6
SCALE = D**-0.5
SHIFT = 2.0
WS = 64.0   # Wq/Wk pre-scale (cancels in RMS norm)
VS = 32.0   # Wv pre-scale == ctx scale alpha (fp8 range)
BETA = 64.0  # Wo pre-scale (fp8 range); host divides by VS*BETA

P = 128
H = D // 2
KP = HID // 256     # 8 K-pair chunks of 256
NTR = T // 512      # 4 T-ranges of 512

F32 = mybir.dt.float32
BF16 = mybir.dt.bfloat16
F8 = mybir.dt.float8e4
DR = mybir.MatmulPerfMode.DoubleRow
ACT_EXP = mybir.ActivationFunctionType.Exp
ACT_SQRT = mybir.ActivationFunctionType.Sqrt
ACT_SQUARE = mybir.ActivationFunctionType.Square
MIN = mybir.AluOpType.min
MULT = mybir.AluOpType.mult
SUB = mybir.AluOpType.subtract

QR_ORDER = [0, 1, 2, 3]  # last one takes the bf16-copy output path


def build_nc():
    nc = bacc.Bacc("TRN2", target_bir_lowering=False, debug=False,
                   num_devices=N_CORES)

    # ---- DRAM tensors (names = in_map keys) ----
    xh = nc.dram_tensor("xh", [P, KP, 2, T], F8, kind="ExternalInput")
    xl = nc.dram_tensor("xl", [P, KP, 2, T], F8, kind="ExternalInput")
    wqh = nc.dram_tensor("wqh", [P, KP, 2, HQ * D], F8, kind="ExternalInput")
    wql = nc.dram_tensor("wql", [P, KP, 2, HQ * D], F8, kind="ExternalInput")
    wkh = nc.dram_tensor("wkh", [P, KP, 2, D], F8, kind="ExternalInput")
    wkl = nc.dram_tensor("wkl", [P, KP, 2, D], F8, kind="ExternalInput")
    wvh = nc.dram_tensor("wvh", [P, KP, 2, D], F8, kind="ExternalInput")
    wvl = nc.dram_tensor("wvl", [P, KP, 2, D], F8, kind="ExternalInput")
    woh = nc.dram_tensor("woh", [P, HQ, HID], F8, kind="ExternalInput")
    wol = nc.dram_tensor("wol", [P, HQ, HID], F8, kind="ExternalInput")
    cosT = nc.dram_tensor("cosT", [P, T], BF16, kind="ExternalInput")
    sinT = nc.dram_tensor("sinT", [P, T], BF16, kind="ExternalInput")
    qw = nc.dram_tensor("qw", [P, 1], F32, kind="ExternalInput")
    kw = nc.dram_tensor("kw", [P, 1], F32, kind="ExternalInput")
    masks = nc.dram_tensor("masks", [P, P], BF16, kind="ExternalInput")
    out = nc.dram_tensor("out", [T, HID], BF16, kind="ExternalOutput")

    with tile.TileContext(nc) as tc:
        with (
            tc.tile_pool(name="cst", bufs=1) as cst,
            tc.tile_pool(name="fin", bufs=1) as fin,
        ):
            # ---------- constants / weights resident in SBUF ----------
            xh_sb = cst.tile([P, KP, 2, T], F8)
            xl_sb = cst.tile([P, KP, 2, T], F8)
            wqh_sb = cst.tile([P, KP, 2, HQ * D], F8)
            wql_sb = cst.tile([P, KP, 2, HQ * D], F8)
            wkh_sb = cst.tile([P, KP, 2, D], F8)
            wkl_sb = cst.tile([P, KP, 2, D], F8)
            wvh_sb = cst.tile([P, KP, 2, D], F8)
            wvl_sb = cst.tile([P, KP, 2, D], F8)
            woh_sb = cst.tile([P, HQ, HID], F8)
            wol_sb = cst.tile([P, HQ, HID], F8)
            masks_sb = cst.tile([P, P], BF16)
            cos_sb = cst.tile([P, T], BF16)
            sin_sb = cst.tile([P, T], BF16)
            qw_sb = cst.tile([P, 1], F32)
            kw_sb = cst.tile([P, 1], F32)
            nc.scalar.dma_start(qw_sb[:], qw[:])
            nc.scalar.dma_start(kw_sb[:], kw[:])
            ones_b = cst.tile([P, 1], BF16)
            nc.vector.memset(ones_b[:], 1.0)
            # DoubleRow ldweights requires the 2-plane dim step % 16 == 0
            w1_8 = cst.tile([P, 2, 16], F8)
            nc.vector.memset(w1_8[:], 1.0)
            shift_sb = cst.tile([P, 1], F32)
            nc.vector.memset(shift_sb[:], -SHIFT)

            # post RMS+RoPE q/k in bf16 (d on partitions)
            qT = [fin.tile([P, T], BF16, name=f"qT_{s}") for s in range(3)]
            # V (VS x): fp8 plane-pairs (plane = st parity) + bf16 st 0-3
            vp = fin.tile([P, T // 256, 2, D], F8)
            v0b = fin.tile([P, 4, D], BF16)
            # normalized ctx (VS x), fp8 hi/lo, plane = head
            ctxC = fin.tile([P, HQ, T], F8)
            ctxL = fin.tile([P, HQ, T], F8)

            # ==== SBUF pools span both phases (a reopened pool would reuse
            # phase A's region and serialize phase C behind its last reader)
            with (
                tc.tile_pool(name="tmpp", bufs=4) as tmpp,
                tc.tile_pool(name="atp", bufs=6) as atp,
                tc.tile_pool(name="adp", bufs=4) as adp,
                tc.tile_pool(name="cfp", bufs=2) as cfp,
                tc.tile_pool(name="otp", bufs=8) as otp,
                tc.tile_pool(name="attp", bufs=4) as attp,
            ):
              # ==== Phase A (split-fp8 DR projections) + B (RMS+RoPE) ====
              deferred = []
              with (
                tc.tile_pool(name="psA", bufs=4, space="PSUM") as psA,
                tc.tile_pool(name="psV", bufs=2, space="PSUM") as psV,
                tc.tile_pool(name="psB", bufs=2, space="PSUM") as psB,
              ):
                for tr in range(NTR):
                    ts = slice(tr * 512, (tr + 1) * 512)
                    if tr == 0:
                        # x is fully SBUF-resident. All phase-A-critical
                        # loads go through ONE queue (SP) in exact need
                        # order -- DMA_ENGINES serves transfers in DGE
                        # arrival order, so multiple queues let late bulk
                        # loads cut ahead of soon-needed weights. The tr0
                        # chunks are kp-split so the PE can start early.
                        nc.sync.dma_start(wqh_sb[:], wqh[:])
                        nc.sync.dma_start(xh_sb[:, 0:4, :, 0:512],
                                          xh[:, 0:4, :, 0:512])
                        nc.sync.dma_start(xh_sb[:, 4:8, :, 0:512],
                                          xh[:, 4:8, :, 0:512])
                        nc.sync.dma_start(wkh_sb[:], wkh[:])
                        nc.sync.dma_start(wvh_sb[:], wvh[:])
                        nc.sync.dma_start(wql_sb[:], wql[:])
                        nc.sync.dma_start(wkl_sb[:], wkl[:])
                        nc.sync.dma_start(wvl_sb[:], wvl[:])
                        nc.sync.dma_start(xl_sb[:, 0:4, :, 0:512],
                                          xl[:, 0:4, :, 0:512])
                        nc.sync.dma_start(xl_sb[:, 4:8, :, 0:512],
                                          xl[:, 4:8, :, 0:512])
                        for r in range(1, NTR):
                            rs = slice(r * 512, (r + 1) * 512)
                            nc.sync.dma_start(xh_sb[:, :, :, rs],
                                              xh[:, :, :, rs])
                            nc.sync.dma_start(xl_sb[:, :, :, rs],
                                              xl[:, :, :, rs])
                        # cos/sin at the tail of the SP stream: program
                        # order precedes tr0's RoPE reads (required for dep
                        # tracking) but the transfers queue after the x
                        # stream (RoPE tolerates late cos/sin)
                        nc.sync.dma_start(cos_sb[:], cosT[:])
                        nc.sync.dma_start(sin_sb[:], sinT[:])
                    if tr == 2:
                        nc.gpsimd.dma_start(masks_sb[:], masks[:])
                        nc.gpsimd.dma_start(woh_sb[:], woh[:])
                        nc.gpsimd.dma_start(wol_sb[:], wol[:])

                    terms = ((wqh_sb, wkh_sb, wvh_sb, xh_sb),
                             (wql_sb, wkl_sb, wvl_sb, xh_sb),
                             (wqh_sb, wkh_sb, wvh_sb, xl_sb))

                    # --- projections q0, q1, k + direct-transposed v ---
                    # tr 0 runs term-outer so the lo-weight / x-lo DMAs are
                    # needed as late as possible while the stream warms up;
                    # later trs run s-outer (one psA tile in flight each)
                    psv = psV.tile([P, 4, D], F32, name="psv")

                    def v_mm(wv_t, xt_, kp, j, first, last):
                        jts = slice(tr * 512 + j * P,
                                    tr * 512 + (j + 1) * P)
                        nc.tensor.matmul(
                            psv[:, j, :], xt_[:, kp, :, jts],
                            wv_t[:, kp, :, :],
                            perf_mode=DR, start=first, stop=last,
                        )

                    def qk_mm(ps, s, wq_t, wk_t, xt_, kp, first, last):
                        wt = wq_t if s < 2 else wk_t
                        cs = slice(s * D, (s + 1) * D) if s < 2 \
                            else slice(0, D)
                        nc.tensor.matmul(
                            ps[:], wt[:, kp, :, cs], xt_[:, kp, :, ts],
                            perf_mode=DR, start=first, stop=last,
                        )

                    if tr == 0:
                        raw = [psA.tile([P, 512], F32, name="psA_t")
                               for _ in range(3)]
                        for t_, (wq_t, wk_t, wv_t, xt_) in enumerate(terms):
                            for kp in range(KP):
                                first = t_ == 0 and kp == 0
                                last = t_ == 2 and kp == KP - 1
                                for s in range(3):
                                    qk_mm(raw[s], s, wq_t, wk_t, xt_,
                                          kp, first, last)
                    else:
                        raw = []
                        for s in range(3):
                            ps = psA.tile([P, 512], F32, name="psA_t")
                            i = 0
                            for wq_t, wk_t, _, xt_ in terms:
                                for kp in range(KP):
                                    qk_mm(ps, s, wq_t, wk_t, xt_, kp,
                                          i == 0, i == 3 * KP - 1)
                                    i += 1
                            raw.append(ps)
                    # one accumulation group for the whole bank: the
                    # first start lazily zeroes the full 2KB zero region
                    for j in range(4):
                        i = 0
                        for _, _, wv_t, xt_ in terms:
                            for kp in range(KP):
                                v_mm(wv_t, xt_, kp, j,
                                     j == 0 and i == 0,
                                     j == 3 and i == 3 * KP - 1)
                                i += 1
                    for j in range(4):
                        st = 4 * tr + j
                        nc.vector.tensor_copy(vp[:, st // 2, st % 2, :],
                                              psv[:, j, :])
                        if tr == 0:
                            nc.vector.tensor_copy(v0b[:, st, :], psv[:, j, :])

                    # --- B: RMS norm + RoPE for q0, q1, k (bf16) ---
                    # tr3's DVE/Pool back-half (bcast+stt+RoPE) is deferred
                    # past qr0's attention so phase C's DVE work isn't
                    # queued behind it (qr3 needs tr3's qT much later)
                    for s in range(3):
                        w_sb = qw_sb if s < 2 else kw_sb
                        # free the psA bank early for the next projection /
                        # the phase-C PSUM pools (Pool has slack here)
                        src = tmpp.tile([P, 512], F32, name="src")
                        nc.scalar.copy(src[:], raw[s][:])
                        sq = tmpp.tile([P, 512], BF16, name="sq")
                        nc.scalar.activation(sq[:], src[:], ACT_SQUARE)
                        ssum = psB.tile([1, 512], F32, name="ssum")
                        nc.tensor.matmul(ssum[:], ones_b[:], sq[:],
                                         start=True, stop=True)
                        # src holds 64*q; host folds sqrt(D) into q/k norm
                        # weights, eps is negligible vs ssum ~ 3e5
                        rstd = tmpp.tile([1, 512], F32, name="rstd")
                        nc.scalar.activation(rstd[:], ssum[:], ACT_SQRT)
                        rinv = tmpp.tile([1, 512], F32, name="rinv")
                        nc.vector.reciprocal_approx_fast(rinv[:], rstd[:])

                        def back_half(s=s, w_sb=w_sb, src=src, rinv=rinv,
                                      ts=ts):
                            rb = tmpp.tile([P, 512], F32, name="rb")
                            nc.gpsimd.partition_broadcast(rb[:], rinv[:])
                            nq = tmpp.tile([P, 512], BF16, name="nq")
                            nc.vector.scalar_tensor_tensor(
                                nq[:], src[:], w_sb[:], rb[:], MULT, MULT,
                            )
                            # RoPE: sin pre-rolled by 64 partitions with the
                            # rotate-half sign folded in; one full-width add
                            psn = tmpp.tile([P, 512], BF16, name="psn")
                            nc.vector.tensor_mul(psn[0:H, :], nq[H:D, :],
                                                 sin_sb[H:D, ts])
                            nc.vector.tensor_mul(psn[H:D, :], nq[0:H, :],
                                                 sin_sb[0:H, ts])
                            pc = tmpp.tile([P, 512], BF16, name="pc")
                            nc.vector.tensor_mul(pc[:], nq[:], cos_sb[:, ts])
                            nc.vector.tensor_add(qT[s][:, ts], pc[:], psn[:])

                        if tr == NTR - 1:
                            deferred.append(back_half)
                        else:
                            back_half()

              # ===== Phase C: attention + o_proj =====
              with (
                tc.tile_pool(name="psP", bufs=2, space="PSUM") as psP,
                tc.tile_pool(name="psCX", bufs=2, space="PSUM") as psCX,
                tc.tile_pool(name="psSM", bufs=1, space="PSUM") as psSM,
                tc.tile_pool(name="psD", bufs=3, space="PSUM") as psD,
              ):
                kT = qT[2]
                pending = []
                tail_dmas = []

                def make_task(qr, tt, nr, idx, pools=None, tail=False,
                              dma_list=None):
                    abs_tt = 4 * qr + tt
                    tts = slice(abs_tt * P, (abs_tt + 1) * P)
                    ns = slice(nr * 512, (nr + 1) * 512)

                    def go():
                        if pools is None:
                            ps = psD.tile([P, 512], F32, name="psD_t")
                        else:
                            # tail: attention PSUM banks are free; borrow
                            # them so the last o_proj burst isn't throttled
                            # by psD recycling
                            pool, tag = pools[idx % len(pools)]
                            ps = pool.tile([P, 512], F32, name=tag)
                        for i, (cs, ws) in enumerate(
                                ((ctxC, woh_sb), (ctxL, woh_sb),
                                 (ctxC, wol_sb))):
                            nc.tensor.matmul(
                                ps[:], cs[:, :, tts], ws[:, :, ns],
                                perf_mode=DR,
                                start=(i == 0), stop=(i == 2))
                        ot = otp.tile([P, 512], BF16, name="ot")
                        # GPSIMD cannot read PSUM; during attention the
                        # copies go Act-heavy (DVE is loaded), in the tail
                        # burst they alternate evenly with the idle DVE
                        if tail and idx % 2 == 0:
                            nc.scalar.copy(ot[:], ps[:])
                        else:
                            nc.vector.tensor_copy(ot[:], ps[:])
                        if dma_list is not None:
                            # tail burst: DMAs issued in a second pass on
                            # two queues, after the copies are in flight
                            dma_list.append((ot, tts, ns))
                        else:
                            # out DMAs only on sync: a dma_start blocks
                            # its issuing engine's SEQ until the copy
                            # dependency resolves, so compute queues must
                            # not carry them
                            nc.sync.dma_start(out[tts, ns], ot[:])
                    return go

                def emit_fill(k):
                    for _ in range(min(k, len(pending))):
                        pending.pop(0)()

                def attn_part(qr, h, qoff, qlen):
                    """Attention for queries [qr*512+qoff, +qlen) of head h."""
                    qs = slice(qr * 512 + qoff, qr * 512 + qoff + qlen)
                    at_dt = BF16 if qr == 0 else F8
                    n_off = 2 * qr
                    ctx_ps = psCX.tile([P, 512], F32, name="ctx_ps")
                    sums_t = psSM.tile([1, 512], F32, name="sums_t")
                    sums_ps = sums_t[:]
                    j0, j1 = qoff // P, (qoff + qlen) // P
                    # --- fully-causal pairs below the diagonal block ---
                    for pi in range(n_off):
                        at = atp.tile([P, 2, 512], at_dt, name="at")
                        for half in range(2):
                            st = 2 * pi + half
                            s_ps = psP.tile([P, 512], F32, name="s_t")
                            nc.tensor.matmul(
                                s_ps[:, 0:qlen], kT[:, st * P:(st + 1) * P],
                                qT[h][:, qs], start=True, stop=True)
                            nc.scalar.activation(
                                at[:, half, 0:qlen], s_ps[:, 0:qlen],
                                ACT_EXP, scale=SCALE, bias=shift_sb[:])
                        nc.tensor.matmul(
                            ctx_ps[:, 0:qlen], vp[:, pi, :, :],
                            at[:, :, 0:qlen],
                            perf_mode=DR, start=(pi == 0), stop=False)
                        nc.tensor.matmul(
                            sums_ps[:, 0:qlen], w1_8[:, :, 0:1],
                            at[:, :, 0:qlen],
                            perf_mode=DR, start=(pi == 0), stop=False)
                    # --- diagonal block, 128-query granular ---
                    for j in range(j0, j1):
                        jr = j - j0
                        jsl = slice(jr * P, (jr + 1) * P)
                        qjs = slice(qr * 512 + j * P, qr * 512 + (j + 1) * P)
                        sd = psP.tile([P, 4, P], F32, name="s_t")
                        for i in range(j + 1):
                            st = 4 * qr + i
                            nc.tensor.matmul(
                                sd[:, i, :], kT[:, st * P:(st + 1) * P],
                                qT[h][:, qjs], start=True, stop=True)
                        ad = adp.tile([P, 4, P], at_dt, name="ad")
                        nc.scalar.activation(
                            ad[:, 0:j + 1, :], sd[:, 0:j + 1, :],
                            ACT_EXP, scale=SCALE, bias=shift_sb[:])
                        # only the true-diagonal tile needs masking
                        nc.vector.tensor_tensor(
                            ad[:, j, :], ad[:, j, :], masks_sb[:], MIN)
                        if qr == 0:
                            # one group: first start zeroes the whole bank
                            for i in range(j + 1):
                                st_ = j == j0 and i == 0
                                fin0 = j == j1 - 1 and i == j
                                nc.tensor.matmul(
                                    ctx_ps[:, jsl], v0b[:, i, :],
                                    ad[:, i, :], start=st_, stop=fin0)
                                nc.tensor.matmul(
                                    sums_ps[:, jsl], ones_b[:],
                                    ad[:, i, :], start=st_, stop=fin0)
                        else:
                            # one group (opened by off-diag pi==0 over the
                            # full query range): stop only on the final
                            # matmul of the last diag subtile
                            fin = j == j1 - 1
                            np_full = (j + 1) // 2
                            for p_ in range(np_full):
                                last_ = (j % 2 == 1) and (p_ == np_full - 1)
                                nc.tensor.matmul(
                                    ctx_ps[:, jsl],
                                    vp[:, 2 * qr + p_, :, :],
                                    ad[:, 2 * p_:2 * p_ + 2, :],
                                    perf_mode=DR, start=False,
                                    stop=(fin and last_))
                                nc.tensor.matmul(
                                    sums_ps[:, jsl], w1_8[:, :, 0:1],
                                    ad[:, 2 * p_:2 * p_ + 2, :],
                                    perf_mode=DR, start=False,
                                    stop=(fin and last_))
                            if j % 2 == 0:  # odd plane count: tail tile
                                nc.tensor.matmul(
                                    ctx_ps[:, jsl],
                                    vp[:, 2 * qr + j // 2, j % 2, :],
                                    ad[:, j, :], start=False, stop=fin)
                                nc.tensor.matmul(
                                    sums_ps[:, jsl], w1_8[:, 0, 0:1],
                                    ad[:, j, :], start=False, stop=fin)
                        emit_fill(1)
                    # --- normalize + fp8 hi/lo split of ctx ---
                    recip = attp.tile([1, 512], F32, name="recip")
                    nc.vector.reciprocal_approx_fast(
                        recip[:, 0:qlen], sums_ps[:, 0:qlen])
                    rb = attp.tile([P, 512], F32, name="rbc")
                    nc.gpsimd.partition_broadcast(
                        rb[:, 0:qlen], recip[:, 0:qlen])
                    cf = cfp.tile([P, 512], F32, name="cf")
                    nc.vector.tensor_mul(
                        cf[:, 0:qlen], ctx_ps[:, 0:qlen], rb[:, 0:qlen])
                    nc.gpsimd.tensor_copy(ctxC[:, h, qs], cf[:, 0:qlen])
                    nc.vector.scalar_tensor_tensor(
                        ctxL[:, h, qs], cf[:, 0:qlen], 1.0, ctxC[:, h, qs],
                        MULT, SUB)
                    emit_fill(3)

                for qi, qr in enumerate(QR_ORDER):
                    last = qi == len(QR_ORDER) - 1
                    # the last block runs as two query halves so half of
                    # its o_proj overlaps the second half's attention
                    parts = ((0, 256), (256, 256)) if last else ((0, 512),)
                    for qoff, qlen in parts:
                        for h in range(HQ):
                            attn_part(qr, h, qoff, qlen)
                        fin_part = last and qoff > 0
                        tailp = [(psD, "psD_t"), (psP, "s_t"),
                                 (psCX, "ctx_ps")] if fin_part else None
                        dml = tail_dmas if fin_part else None
                        for tt in range(qoff // P, (qoff + qlen) // P):
                            for nr in range(4):
                                pending.append(
                                    make_task(qr, tt, nr, 4 * tt + nr,
                                              pools=tailp, tail=last,
                                              dma_list=dml))
                    if qi == 0:
                        for fn in deferred:
                            fn()
                        deferred.clear()
                    if last:
                        emit_fill(len(pending))
                        for i, (ot, tts, ns) in enumerate(tail_dmas):
                            eng = nc.sync if i % 2 == 0 else nc.scalar
                            eng.dma_start(out[tts, ns], ot[:])

    nc.compile()
    return nc


_NC_CACHE = None


def get_nc():
    global _NC_CACHE
    if _NC_CACHE is None:
        _NC_CACHE = build_nc()
    return _NC_CACHE


F8NP = ml_dtypes.float8_e4m3
BF16NP = ml_dtypes.bfloat16


def _fold_hid(a):
    """[HID, C] -> [P, KP, 2, C] with hid = kp*256 + pl*128 + p."""
    c = a.shape[1]
    return np.ascontiguousarray(
        a.reshape(KP, 2, P, c).transpose(2, 0, 1, 3))


def _split8(a):
    hi = a.astype(F8NP)
    lo = (a - hi.astype(np.float32)).astype(F8NP)
    return hi, lo


def make_in_maps(x, cos, sin, Wq, Wk, Wv, Wo, q_norm_w, k_norm_w):
    x = np.asarray(x, dtype=np.float32).reshape(T, HID)
    xf = _fold_hid(np.ascontiguousarray(x.T).reshape(HID, T))
    xh, xl = _split8(xf)
    cosb = np.ascontiguousarray(
        np.asarray(cos, np.float32).T).astype(BF16NP)
    # rolled by 64 with rotate-half signs folded in:
    # psn[0:64] (subtracted in ref) uses rows 64:128 -> negate those rows
    sr = np.roll(np.asarray(sin, np.float32).T, 64, axis=0)
    sr[64:, :] *= -1.0
    sinb = np.ascontiguousarray(sr).astype(BF16NP)
    # sqrt(D) folded here: kernel computes rinv = (sum (64 q)^2)^-0.5
    sqd = np.float32(np.sqrt(D))
    qwa = np.ascontiguousarray(
        np.asarray(q_norm_w, np.float32).reshape(D, 1) * sqd)
    kwa = np.ascontiguousarray(
        np.asarray(k_norm_w, np.float32).reshape(D, 1) * sqd)
    si = np.arange(P)[:, None]
    qi = np.arange(P)[None, :]
    masks = np.where(si <= qi, 240.0, 0.0).astype(BF16NP)
    Wq = np.asarray(Wq, np.float32) * WS
    Wk = np.asarray(Wk, np.float32) * WS
    Wv = np.asarray(Wv, np.float32) * VS
    Wo = np.asarray(Wo, np.float32) * BETA
    in_maps = []
    for c in range(N_CORES):
        wqh_, wql_ = _split8(_fold_hid(Wq[:, c * HQ * D:(c + 1) * HQ * D]))
        wkh_, wkl_ = _split8(_fold_hid(Wk[:, c * D:(c + 1) * D]))
        wvh_, wvl_ = _split8(_fold_hid(Wv[:, c * D:(c + 1) * D]))
        wo_ = np.ascontiguousarray(
            Wo[c * HQ * D:(c + 1) * HQ * D, :].reshape(HQ, P, HID)
            .transpose(1, 0, 2))
        woh_, wol_ = _split8(wo_)
        in_maps.append({
            "xh": xh, "xl": xl,
            "wqh": wqh_, "wql": wql_,
            "wkh": wkh_, "wkl": wkl_,
            "wvh": wvh_, "wvl": wvl_,
            "woh": woh_, "wol": wol_,
            "cosT": cosb, "sinT": sinb,
            "qw": qwa, "kw": kwa,
            "masks": masks,
        })
    return in_maps


def kernel(x, cos, sin, Wq, Wk, Wv, Wo, q_norm_w, k_norm_w):
    nc = get_nc()
    in_maps = make_in_maps(x, cos, sin, Wq, Wk, Wv, Wo, q_norm_w, k_norm_w)
    res = run_bass_kernel_spmd(nc, in_maps, core_ids=list(range(N_CORES)))
    acc = np.zeros((T, HID), dtype=np.float32)
    for c in range(N_CORES):
        acc += np.asarray(res.results[c]["out"], np.float32)
    acc *= 1.0 / (VS * BETA)
    return acc.reshape(1, T, HID)


# revision 86
# speedup vs baseline: 1.1938x; 1.0000x over previous
"""GQA attention block (B=1, T=2048, HID=2048, NQ=16, NKV=8, D=128) on 8 TRN2
NeuronCores.

Sharding: tensor-parallel over heads. Core c owns q-heads {2c, 2c+1} and
kv-head c. The 8 partial outputs are summed on the host (scaled 1/(VS*BETA)).

v2 speed strategy (tuned against the TimelineSim cost model, validated on
device + interpreter + f64 reference):
  - projections: 3-term split-fp8 (xh*wh + xl*wh + xh*wl) with K=256
    DoubleRow matmuls (0.5 cyc/row in the cost model). Wq/Wk pre-scaled by
    WS=64 (cancels through RMS norm), Wv by VS=32. x is SBUF-resident,
    streamed on one queue in exact need order (DMA_ENGINES serves in DGE
    arrival order). tr0 runs term-outer so lo-weight/x-lo land later.
  - V is projected directly transposed ([t, d] tiles, stationary = x
    chunk), so no PE transposes are needed.
  - q/k: RMS-norm (Act sq/sqrt + DVE recip; sqrt(D) folded into the host
    norm weights, eps negligible) + RoPE on DVE in bf16. tr3's DVE
    back-half is deferred past qr0's attention (qr3 needs it much later).
  - attention: at = exp(score/sqrt(D) - 2), fp8 for q-rows >= 512, bf16
    below. The diagonal 512x512 block is 128-query granular: only needed
    key tiles are computed and only the true-diagonal tile is min-masked
    (mask in {0, 240}: min(sat, 0) = 0 kills acausal fp8-overflowed exp).
    One PSUM accumulation group per (block, head): single start lazily
    zeroes the 2KB region, single stop at the end.
  - denominators: ones-stationary matmuls accumulated alongside ctx.
  - o_proj: 3-term split-fp8 DoubleRow with BOTH heads packed into K=256;
    ctx is normalized then split hi/lo (DVE mul + Pool fp8 copy + DVE
    subtract); Wo pre-scaled by BETA=64, ctx carries VS=32, host divides
    by 2048. PSUM -> bf16 out copies run on DVE (Act is exp-bound),
    alternating with Act only in the tail burst; out DMAs issue on sync
    only (a dma_start blocks its queue's SEQ until the copy resolves).
  - schedule: q-blocks in order 0..3; each block's 16 o_proj tiles are
    queued and interleaved as PE filler into the NEXT block's attention
    (concentrated at diag/normalize points where Act latency would
    otherwise stall the PE). The last block runs as two 256-query halves
    (half the tail overlaps the second half's attention), and its tiles
    borrow the freed attention PSUM banks.
"""

import sys

sys.path.insert(0, "/opt/trn_rl_repo")

import numpy as np
import ml_dtypes

import concourse.bass as bass  # noqa: F401  (bass must import before tile)
import concourse.mybir as mybir
import concourse.tile as tile
from concourse import bacc
from concourse.bass_utils import run_bass_kernel_spmd

N_CORES = 8
T = 2048
HID = 2048
NQ, NKV, D = 16, 8, 128
HQ = NQ // N_CORES  # q heads per core = 2
EPS = 1e-6
SCALE = D**-0.5
SHIFT = 2.0
WS = 64.0   # Wq/Wk pre-scale (cancels in RMS norm)
VS = 32.0   # Wv pre-scale == ctx scale alpha (fp8 range)
BETA = 64.0  # Wo pre-scale (fp8 range); host divides by VS*BETA

P = 128
H = D // 2
KP = HID // 256     # 8 K-pair chunks of 256
NTR = T // 512      # 4 T-ranges of 512

F32 = mybir.dt.float32
BF16 = mybir.dt.bfloat16
F8 = mybir.dt.float8e4
DR = mybir.MatmulPerfMode.DoubleRow
ACT_EXP = mybir.ActivationFunctionType.Exp
ACT_SQRT = mybir.ActivationFunctionType.Sqrt
ACT_SQUARE = mybir.ActivationFunctionType.Square
MIN = mybir.AluOpType.min
MULT = mybir.AluOpType.mult
SUB = mybir.AluOpType.subtract

QR_ORDER = [0, 1, 2, 3]  # last one takes the bf16-copy output path


def build_nc():
    nc = bacc.Bacc("TRN2", target_bir_lowering=False, debug=False,
                   num_devices=N_CORES)

    # ---- DRAM tensors (names = in_map keys) ----
    xh = nc.dram_tensor("xh", [P, KP, 2, T], F8, kind="ExternalInput")
    xl = nc.dram_tensor("xl", [P, KP, 2, T], F8, kind="ExternalInput")
    wqh = nc.dram_tensor("wqh", [P, KP, 2, HQ * D], F8, kind="ExternalInput")
    wql = nc.dram_tensor("wql", [P, KP, 2, HQ * D], F8, kind="ExternalInput")
    wkh = nc.dram_tensor("wkh", [P, KP, 2, D], F8, kind="ExternalInput")
    wkl = nc.dram_tensor("wkl", [P, KP, 2, D], F8, kind="ExternalInput")
    wvh = nc.dram_tensor("wvh", [P, KP, 2, D], F8, kind="ExternalInput")
    wvl = nc.dram_tensor("wvl", [P, KP, 2, D], F8, kind="ExternalInput")
    woh = nc.dram_tensor("woh", [P, HQ, HID], F8, kind="ExternalInput")
    wol = nc.dram_tensor("wol", [P, HQ, HID], F8, kind="ExternalInput")
    cosT = nc.dram_tensor("cosT", [P, T], BF16, kind="ExternalInput")
    sinT = nc.dram_tensor("sinT", [P, T], BF16, kind="ExternalInput")
    qw = nc.dram_tensor("qw", [P, 1], F32, kind="ExternalInput")
    kw = nc.dram_tensor("kw", [P, 1], F32, kind="ExternalInput")
    masks = nc.dram_tensor("masks", [P, P], BF16, kind="ExternalInput")
    out = nc.dram_tensor("out", [T, HID], BF16, kind="ExternalOutput")

    with tile.TileContext(nc) as tc:
        with (
            tc.tile_pool(name="cst", bufs=1) as cst,
            tc.tile_pool(name="fin", bufs=1) as fin,
        ):
            # ---------- constants / weights resident in SBUF ----------
            xh_sb = cst.tile([P, KP, 2, T], F8)
            xl_sb = cst.tile([P, KP, 2, T], F8)
            wqh_sb = cst.tile([P, KP, 2, HQ * D], F8)
            wql_sb = cst.tile([P, KP, 2, HQ * D], F8)
            wkh_sb = cst.tile([P, KP, 2, D], F8)
            wkl_sb = cst.tile([P, KP, 2, D], F8)
            wvh_sb = cst.tile([P, KP, 2, D], F8)
            wvl_sb = cst.tile([P, KP, 2, D], F8)
            woh_sb = cst.tile([P, HQ, HID], F8)
            wol_sb = cst.tile([P, HQ, HID], F8)
            masks_sb = cst.tile([P, P], BF16)
            cos_sb = cst.tile([P, T], BF16)
            sin_sb = cst.tile([P, T], BF16)
            qw_sb = cst.tile([P, 1], F32)
            kw_sb = cst.tile([P, 1], F32)
            nc.scalar.dma_start(qw_sb[:], qw[:])
            nc.scalar.dma_start(kw_sb[:], kw[:])
            ones_b = cst.tile([P, 1], BF16)
            nc.vector.memset(ones_b[:], 1.0)
            # DoubleRow ldweights requires the 2-plane dim step % 16 == 0
            w1_8 = cst.tile([P, 2, 16], F8)
            nc.vector.memset(w1_8[:], 1.0)
            shift_sb = cst.tile([P, 1], F32)
            nc.vector.memset(shift_sb[:], -SHIFT)

            # post RMS+RoPE q/k in bf16 (d on partitions)
            qT = [fin.tile([P, T], BF16, name=f"qT_{s}") for s in range(3)]
            # V (VS x): fp8 plane-pairs (plane = st parity) + bf16 st 0-3
            vp = fin.tile([P, T // 256, 2, D], F8)
            v0b = fin.tile([P, 4, D], BF16)
            # normalized ctx (VS x), fp8 hi/lo, plane = head
            ctxC = fin.tile([P, HQ, T], F8)
            ctxL = fin.tile([P, HQ, T], F8)

            # ==== SBUF pools span both phases (a reopened pool would reuse
            # phase A's region and serialize phase C behind its last reader)
            with (
                tc.tile_pool(name="tmpp", bufs=4) as tmpp,
                tc.tile_pool(name="atp", bufs=6) as atp,
                tc.tile_pool(name="adp", bufs=4) as adp,
                tc.tile_pool(name="cfp", bufs=2) as cfp,
                tc.tile_pool(name="otp", bufs=8) as otp,
                tc.tile_pool(name="attp", bufs=4) as attp,
            ):
              # ==== Phase A (split-fp8 DR projections) + B (RMS+RoPE) ====
              deferred = []
              with (
                tc.tile_pool(name="psA", bufs=4, space="PSUM") as psA,
                tc.tile_pool(name="psV", bufs=2, space="PSUM") as psV,
                tc.tile_pool(name="psB", bufs=2, space="PSUM") as psB,
              ):
                for tr in range(NTR):
                    ts = slice(tr * 512, (tr + 1) * 512)
                    if tr == 0:
                        # x is fully SBUF-resident. All phase-A-critical
                        # loads go through ONE queue (SP) in exact need
                        # order -- DMA_ENGINES serves transfers in DGE
                        # arrival order, so multiple queues let late bulk
                        # loads cut ahead of soon-needed weights. The tr0
                        # chunks are kp-split so the PE can start early.
                        nc.sync.dma_start(wqh_sb[:], wqh[:])
                        nc.sync.dma_start(xh_sb[:, 0:4, :, 0:512],
                                          xh[:, 0:4, :, 0:512])
                        nc.sync.dma_start(xh_sb[:, 4:8, :, 0:512],
                                          xh[:, 4:8, :, 0:512])
                        nc.sync.dma_start(wkh_sb[:], wkh[:])
                        nc.sync.dma_start(wvh_sb[:], wvh[:])
                        nc.sync.dma_start(wql_sb[:], wql[:])
                        nc.sync.dma_start(wkl_sb[:], wkl[:])
                        nc.sync.dma_start(wvl_sb[:], wvl[:])
                        nc.sync.dma_start(xl_sb[:, 0:4, :, 0:512],
                                          xl[:, 0:4, :, 0:512])
                        nc.sync.dma_start(xl_sb[:, 4:8, :, 0:512],
                                          xl[:, 4:8, :, 0:512])
                        for r in range(1, NTR):
                            rs = slice(r * 512, (r + 1) * 512)
                            nc.sync.dma_start(xh_sb[:, :, :, rs],
                                              xh[:, :, :, rs])
                            nc.sync.dma_start(xl_sb[:, :, :, rs],
                                              xl[:, :, :, rs])
                        # cos/sin at the tail of the SP stream: program
                        # order precedes tr0's RoPE reads (required for dep
                        # tracking) but the transfers queue after the x
                        # stream (RoPE tolerates late cos/sin)
                        nc.sync.dma_start(cos_sb[:], cosT[:])
                        nc.sync.dma_start(sin_sb[:], sinT[:])
                    if tr == 2:
                        nc.gpsimd.dma_start(masks_sb[:], masks[:])
                        nc.gpsimd.dma_start(woh_sb[:], woh[:])
                        nc.gpsimd.dma_start(wol_sb[:], wol[:])

                    terms = ((wqh_sb, wkh_sb, wvh_sb, xh_sb),
                             (wql_sb, wkl_sb, wvl_sb, xh_sb),
                             (wqh_sb, wkh_sb, wvh_sb, xl_sb))

                    # --- projections q0, q1, k + direct-transposed v ---
                    # tr 0 runs term-outer so the lo-weight / x-lo DMAs are
                    # needed as late as possible while the stream warms up;
                    # later trs run s-outer (one psA tile in flight each)
                    psv = psV.tile([P, 4, D], F32, name="psv")

                    def v_mm(wv_t, xt_, kp, j, first, last):
                        jts = slice(tr * 512 + j * P,
                                    tr * 512 + (j + 1) * P)
                        nc.tensor.matmul(
                            psv[:, j, :], xt_[:, kp, :, jts],
                            wv_t[:, kp, :, :],
                            perf_mode=DR, start=first, stop=last,
                        )

                    def qk_mm(ps, s, wq_t, wk_t, xt_, kp, first, last):
                        wt = wq_t if s < 2 else wk_t
                        cs = slice(s * D, (s + 1) * D) if s < 2 \
                            else slice(0, D)
                        nc.tensor.matmul(
                            ps[:], wt[:, kp, :, cs], xt_[:, kp, :, ts],
                            perf_mode=DR, start=first, stop=last,
                        )

                    if tr == 0:
                        raw = [psA.tile([P, 512], F32, name="psA_t")
                               for _ in range(3)]
                        for t_, (wq_t, wk_t, wv_t, xt_) in enumerate(terms):
                            for kp in range(KP):
                                first = t_ == 0 and kp == 0
                                last = t_ == 2 and kp == KP - 1
                                for s in range(3):
                                    qk_mm(raw[s], s, wq_t, wk_t, xt_,
                                          kp, first, last)
                    else:
                        raw = []
                        for s in range(3):
                            ps = psA.tile([P, 512], F32, name="psA_t")
                            i = 0
                            for wq_t, wk_t, _, xt_ in terms:
                                for kp in range(KP):
                                    qk_mm(ps, s, wq_t, wk_t, xt_, kp,
                                          i == 0, i == 3 * KP - 1)
                                    i += 1
                            raw.append(ps)
                    # one accumulation group for the whole bank: the
                    # first start lazily zeroes the full 2KB zero region
                    for j in range(4):
                        i = 0
                        for _, _, wv_t, xt_ in terms:
                            for kp in range(KP):
                                v_mm(wv_t, xt_, kp, j,
                                     j == 0 and i == 0,
                                     j == 3 and i == 3 * KP - 1)
                                i += 1
                    for j in range(4):
                        st = 4 * tr + j
                        nc.vector.tensor_copy(vp[:, st // 2, st % 2, :],
                                              psv[:, j, :])
                        if tr == 0:
                            nc.vector.tensor_copy(v0b[:, st, :], psv[:, j, :])

                    # --- B: RMS norm + RoPE for q0, q1, k (bf16) ---
                    # tr3's DVE/Pool back-half (bcast+stt+RoPE) is deferred
                    # past qr0's attention so phase C's DVE work isn't
                    # queued behind it (qr3 needs tr3's qT much later)
                    for s in range(3):
                        w_sb = qw_sb if s < 2 else kw_sb
                        # free the psA bank early for the next projection /
                        # the phase-C PSUM pools (Pool has slack here)
                        src = tmpp.tile([P, 512], F32, name="src")
                        nc.scalar.copy(src[:], raw[s][:])
                        sq = tmpp.tile([P, 512], BF16, name="sq")
                        nc.scalar.activation(sq[:], src[:], ACT_SQUARE)
                        ssum = psB.tile([1, 512], F32, name="ssum")
                        nc.tensor.matmul(ssum[:], ones_b[:], sq[:],
                                         start=True, stop=True)
                        # src holds 64*q; host folds sqrt(D) into q/k norm
                        # weights, eps is negligible vs ssum ~ 3e5
                        rstd = tmpp.tile([1, 512], F32, name="rstd")
                        nc.scalar.activation(rstd[:], ssum[:], ACT_SQRT)
                        rinv = tmpp.tile([1, 512], F32, name="rinv")
                        nc.vector.reciprocal_approx_fast(rinv[:], rstd[:])

                        def back_half(s=s, w_sb=w_sb, src=src, rinv=rinv,
                                      ts=ts):
                            rb = tmpp.tile([P, 512], F32, name="rb")
                            nc.gpsimd.partition_broadcast(rb[:], rinv[:])
                            nq = tmpp.tile([P, 512], BF16, name="nq")
                            nc.vector.scalar_tensor_tensor(
                                nq[:], src[:], w_sb[:], rb[:], MULT, MULT,
                            )
                            # RoPE: sin pre-rolled by 64 partitions with the
                            # rotate-half sign folded in; one full-width add
                            psn = tmpp.tile([P, 512], BF16, name="psn")
                            nc.vector.tensor_mul(psn[0:H, :], nq[H:D, :],
                                                 sin_sb[H:D, ts])
                            nc.vector.tensor_mul(psn[H:D, :], nq[0:H, :],
                                                 sin_sb[0:H, ts])
                            pc = tmpp.tile([P, 512], BF16, name="pc")
                            nc.vector.tensor_mul(pc[:], nq[:], cos_sb[:, ts])
                            nc.vector.tensor_add(qT[s][:, ts], pc[:], psn[:])

                        if tr == NTR - 1:
                            deferred.append(back_half)
                        else:
                            back_half()

              # ===== Phase C: attention + o_proj =====
              with (
                tc.tile_pool(name="psP", bufs=2, space="PSUM") as psP,
                tc.tile_pool(name="psCX", bufs=2, space="PSUM") as psCX,
                tc.tile_pool(name="psSM", bufs=1, space="PSUM") as psSM,
                tc.tile_pool(name="psD", bufs=3, space="PSUM") as psD,
              ):
                kT = qT[2]
                pending = []
                tail_dmas = []

                def make_task(qr, tt, nr, idx, pools=None, tail=False,
                              dma_list=None):
                    abs_tt = 4 * qr + tt
                    tts = slice(abs_tt * P, (abs_tt + 1) * P)
                    ns = slice(nr * 512, (nr + 1) * 512)

                    def go():
                        if pools is None:
                            ps = psD.tile([P, 512], F32, name="psD_t")
                        else:
                            # tail: attention PSUM banks are free; borrow
                            # them so the last o_proj burst isn't throttled
                            # by psD recycling
                            pool, tag = pools[idx % len(pools)]
                            ps = pool.tile([P, 512], F32, name=tag)
                        for i, (cs, ws) in enumerate(
                                ((ctxC, woh_sb), (ctxL, woh_sb),
                                 (ctxC, wol_sb))):
                            nc.tensor.matmul(
                                ps[:], cs[:, :, tts], ws[:, :, ns],
                                perf_mode=DR,
                                start=(i == 0), stop=(i == 2))
                        ot = otp.tile([P, 512], BF16, name="ot")
                        # GPSIMD cannot read PSUM; during attention the
                        # copies go Act-heavy (DVE is loaded), in the tail
                        # burst they alternate evenly with the idle DVE
                        if tail and idx % 2 == 0:
                            nc.scalar.copy(ot[:], ps[:])
                        else:
                            nc.vector.tensor_copy(ot[:], ps[:])
                        if dma_list is not None:
                            # tail burst: DMAs issued in a second pass on
                            # two queues, after the copies are in flight
                            dma_list.append((ot, tts, ns))
                        else:
                            # out DMAs only on sync: a dma_start blocks
                            # its issuing engine's SEQ until the copy
                            # dependency resolves, so compute queues must
                            # not carry them
                            nc.sync.dma_start(out[tts, ns], ot[:])
                    return go

                def emit_fill(k):
                    for _ in range(min(k, len(pending))):
                        pending.pop(0)()

                def attn_part(qr, h, qoff, qlen):
                    """Attention for queries [qr*512+qoff, +qlen) of head h."""
                    qs = slice(qr * 512 + qoff, qr * 512 + qoff + qlen)
                    at_dt = BF16 if qr == 0 else F8
                    n_off = 2 * qr
                    ctx_ps = psCX.tile([P, 512], F32, name="ctx_ps")
                    sums_t = psSM.tile([1, 512], F32, name="sums_t")
                    sums_ps = sums_t[:]
                    j0, j1 = qoff // P, (qoff + qlen) // P
                    # --- fully-causal pairs below the diagonal block ---
                    for pi in range(n_off):
                        at = atp.tile([P, 2, 512], at_dt, name="at")
                        for half in range(2):
                            st = 2 * pi + half
                            s_ps = psP.tile([P, 512], F32, name="s_t")
                            nc.tensor.matmul(
                                s_ps[:, 0:qlen], kT[:, st * P:(st + 1) * P],
                                qT[h][:, qs], start=True, stop=True)
                            nc.scalar.activation(
                                at[:, half, 0:qlen], s_ps[:, 0:qlen],
                                ACT_EXP, scale=SCALE, bias=shift_sb[:])
                        nc.tensor.matmul(
                            ctx_ps[:, 0:qlen], vp[:, pi, :, :],
                            at[:, :, 0:qlen],
                            perf_mode=DR, start=(pi == 0), stop=False)
                        nc.tensor.matmul(
                            sums_ps[:, 0:qlen], w1_8[:, :, 0:1],
                            at[:, :, 0:qlen],
                            perf_mode=DR, start=(pi == 0), stop=False)
                    # --- diagonal block, 128-query granular ---
                    for j in range(j0, j1):
                        emit_fill(1)
                        jr = j - j0
                        jsl = slice(jr * P, (jr + 1) * P)
                        qjs = slice(qr * 512 + j * P, qr * 512 + (j + 1) * P)
                        sd = psP.tile([P, 4, P], F32, name="s_t")
                        for i in range(j + 1):
                            st = 4 * qr + i
                            nc.tensor.matmul(
                                sd[:, i, :], kT[:, st * P:(st + 1) * P],
                                qT[h][:, qjs], start=True, stop=True)
                        ad = adp.tile([P, 4, P], at_dt, name="ad")
                        nc.scalar.activation(
                            ad[:, 0:j + 1, :], sd[:, 0:j + 1, :],
                            ACT_EXP, scale=SCALE, bias=shift_sb[:])
                        # only the true-diagonal tile needs masking
                        nc.vector.tensor_tensor(
                            ad[:, j, :], ad[:, j, :], masks_sb[:], MIN)
                        if qr == 0:
                            # one group: first start zeroes the whole bank
                            for i in range(j + 1):
                                st_ = j == j0 and i == 0
                                fin0 = j == j1 - 1 and i == j
                                nc.tensor.matmul(
                                    ctx_ps[:, jsl], v0b[:, i, :],
                                    ad[:, i, :], start=st_, stop=fin0)
                                nc.tensor.matmul(
                                    sums_ps[:, jsl], ones_b[:],
                                    ad[:, i, :], start=st_, stop=fin0)
                        else:
                            # one group (opened by off-diag pi==0 over the
                            # full query range): stop only on the final
                            # matmul of the last diag subtile
                            fin = j == j1 - 1
                            np_full = (j + 1) // 2
                            for p_ in range(np_full):
                                last_ = (j % 2 == 1) and (p_ == np_full - 1)
                                nc.tensor.matmul(
                                    ctx_ps[:, jsl],
                                    vp[:, 2 * qr + p_, :, :],
                                    ad[:, 2 * p_:2 * p_ + 2, :],
                                    perf_mode=DR, start=False,
                                    stop=(fin and last_))
                                nc.tensor.matmul(
                                    sums_ps[:, jsl], w1_8[:, :, 0:1],
                                    ad[:, 2 * p_:2 * p_ + 2, :],
                                    perf_mode=DR, start=False,
                                    stop=(fin and last_))
                            if j % 2 == 0:  # odd plane count: tail tile
                                nc.tensor.matmul(
                                    ctx_ps[:, jsl],
                                    vp[:, 2 * qr + j // 2, j % 2, :],
                                    ad[:, j, :], start=False, stop=fin)
                                nc.tensor.matmul(
                                    sums_ps[:, jsl], w1_8[:, 0, 0:1],
                                    ad[:, j, :], start=False, stop=fin)
                    # --- normalize + fp8 hi/lo split of ctx ---
                    recip = attp.tile([1, 512], F32, name="recip")
                    nc.vector.reciprocal_approx_fast(
                        recip[:, 0:qlen], sums_ps[:, 0:qlen])
                    rb = attp.tile([P, 512], F32, name="rbc")
                    nc.gpsimd.partition_broadcast(
                        rb[:, 0:qlen], recip[:, 0:qlen])
                    cf = cfp.tile([P, 512], F32, name="cf")
                    nc.vector.tensor_mul(
                        cf[:, 0:qlen], ctx_ps[:, 0:qlen], rb[:, 0:qlen])
                    nc.gpsimd.tensor_copy(ctxC[:, h, qs], cf[:, 0:qlen])
                    nc.vector.scalar_tensor_tensor(
                        ctxL[:, h, qs], cf[:, 0:qlen], 1.0, ctxC[:, h, qs],
                        MULT, SUB)
                    emit_fill(3)

                for qi, qr in enumerate(QR_ORDER):
                    last = qi == len(QR_ORDER) - 1
                    # the last block runs as two query halves so half of
                    # its o_proj overlaps the second half's attention
                    parts = ((0, 256), (256, 256)) if last else ((0, 512),)
                    for qoff, qlen in parts:
                        for h in range(HQ):
                            attn_part(qr, h, qoff, qlen)
                        fin_part = last and qoff > 0
                        tailp = [(psD, "psD_t"), (psP, "s_t"),
                                 (psCX, "ctx_ps")] if fin_part else None
                        dml = tail_dmas if fin_part else None
                        for tt in range(qoff // P, (qoff + qlen) // P):
                            for nr in range(4):
                                pending.append(
                                    make_task(qr, tt, nr, 4 * tt + nr,
                                              pools=tailp, tail=last,
                                              dma_list=dml))
                    if qi == 0:
                        for fn in deferred:
                            fn()
                        deferred.clear()
                    if last:
                        emit_fill(len(pending))
                        for i, (ot, tts, ns) in enumerate(tail_dmas):
                            eng = nc.sync if i % 2 == 0 else nc.scalar
                            eng.dma_start(out[tts, ns], ot[:])

    nc.compile()
    return nc


_NC_CACHE = None


def get_nc():
    global _NC_CACHE
    if _NC_CACHE is None:
        _NC_CACHE = build_nc()
    return _NC_CACHE


F8NP = ml_dtypes.float8_e4m3
BF16NP = ml_dtypes.bfloat16


def _fold_hid(a):
    """[HID, C] -> [P, KP, 2, C] with hid = kp*256 + pl*128 + p."""
    c = a.shape[1]
    return np.ascontiguousarray(
        a.reshape(KP, 2, P, c).transpose(2, 0, 1, 3))


def _split8(a):
    hi = a.astype(F8NP)
    lo = (a - hi.astype(np.float32)).astype(F8NP)
    return hi, lo


def make_in_maps(x, cos, sin, Wq, Wk, Wv, Wo, q_norm_w, k_norm_w):
    x = np.asarray(x, dtype=np.float32).reshape(T, HID)
    xf = _fold_hid(np.ascontiguousarray(x.T).reshape(HID, T))
    xh, xl = _split8(xf)
    cosb = np.ascontiguousarray(
        np.asarray(cos, np.float32).T).astype(BF16NP)
    # rolled by 64 with rotate-half signs folded in:
    # psn[0:64] (subtracted in ref) uses rows 64:128 -> negate those rows
    sr = np.roll(np.asarray(sin, np.float32).T, 64, axis=0)
    sr[64:, :] *= -1.0
    sinb = np.ascontiguousarray(sr).astype(BF16NP)
    # sqrt(D) folded here: kernel computes rinv = (sum (64 q)^2)^-0.5
    sqd = np.float32(np.sqrt(D))
    qwa = np.ascontiguousarray(
        np.asarray(q_norm_w, np.float32).reshape(D, 1) * sqd)
    kwa = np.ascontiguousarray(
        np.asarray(k_norm_w, np.float32).reshape(D, 1) * sqd)
    si = np.arange(P)[:, None]
    qi = np.arange(P)[None, :]
    masks = np.where(si <= qi, 240.0, 0.0).astype(BF16NP)
    Wq = np.asarray(Wq, np.float32) * WS
    Wk = np.asarray(Wk, np.float32) * WS
    Wv = np.asarray(Wv, np.float32) * VS
    Wo = np.asarray(Wo, np.float32) * BETA
    in_maps = []
    for c in range(N_CORES):
        wqh_, wql_ = _split8(_fold_hid(Wq[:, c * HQ * D:(c + 1) * HQ * D]))
        wkh_, wkl_ = _split8(_fold_hid(Wk[:, c * D:(c + 1) * D]))
        wvh_, wvl_ = _split8(_fold_hid(Wv[:, c * D:(c + 1) * D]))
        wo_ = np.ascontiguousarray(
            Wo[c * HQ * D:(c + 1) * HQ * D, :].reshape(HQ, P, HID)
            .transpose(1, 0, 2))
        woh_, wol_ = _split8(wo_)
        in_maps.append({
            "xh": xh, "xl": xl,
            "wqh": wqh_, "wql": wql_,
            "wkh": wkh_, "wkl": wkl_,
            "wvh": wvh_, "wvl": wvl_,
            "woh": woh_, "wol": wol_,
            "cosT": cosb, "sinT": sinb,
            "qw": qwa, "kw": kwa,
            "masks": masks,
        })
    return in_maps


def kernel(x, cos, sin, Wq, Wk, Wv, Wo, q_norm_w, k_norm_w):
    nc = get_nc()
    in_maps = make_in_maps(x, cos, sin, Wq, Wk, Wv, Wo, q_norm_w, k_norm_w)
    res = run_bass_kernel_spmd(nc, in_maps, core_ids=list(range(N_CORES)))
    acc = np.zeros((T, HID), dtype=np.float32)
    for c in range(N_CORES):
        acc += np.asarray(res.results[c]["out"], np.float32)
    acc *= 1.0 / (VS * BETA)
    return acc.reshape(1, T, HID)


# revision 91
# speedup vs baseline: 1.2130x; 1.0161x over previous
"""GQA attention block (B=1, T=2048, HID=2048, NQ=16, NKV=8, D=128) on 8 TRN2
NeuronCores.

Sharding: tensor-parallel over heads. Core c owns q-heads {2c, 2c+1} and
kv-head c. The 8 partial outputs are summed on the host (scaled 1/(VS*BETA)).

v2 speed strategy (tuned against the TimelineSim cost model, validated on
device + interpreter + f64 reference):
  - projections: 3-term split-fp8 (xh*wh + xl*wh + xh*wl) with K=256
    DoubleRow matmuls (0.5 cyc/row in the cost model). Wq/Wk pre-scaled by
    WS=64 (cancels through RMS norm), Wv by VS=32. x is SBUF-resident,
    streamed on one queue in exact need order (DMA_ENGINES serves in DGE
    arrival order). tr0 runs term-outer so lo-weight/x-lo land later.
  - V is projected directly transposed ([t, d] tiles, stationary = x
    chunk), so no PE transposes are needed.
  - q/k: RMS-norm (Act sq/sqrt + DVE recip; sqrt(D) folded into the host
    norm weights, eps negligible) + RoPE on DVE in bf16. tr3's DVE
    back-half is deferred past qr0's attention (qr3 needs it much later).
  - attention: at = exp(score/sqrt(D) - 2), fp8 for q-rows >= 512, bf16
    below. The diagonal 512x512 block is 128-query granular: only needed
    key tiles are computed and only the true-diagonal tile is min-masked
    (mask in {0, 240}: min(sat, 0) = 0 kills acausal fp8-overflowed exp).
    One PSUM accumulation group per (block, head): single start lazily
    zeroes the 2KB region, single stop at the end.
  - denominators: ones-stationary matmuls accumulated alongside ctx.
  - o_proj: 3-term split-fp8 DoubleRow with BOTH heads packed into K=256;
    ctx is normalized then split hi/lo (DVE mul + Pool fp8 copy + DVE
    subtract); Wo pre-scaled by BETA=64, ctx carries VS=32, host divides
    by 2048. PSUM -> bf16 out copies run on DVE (Act is exp-bound),
    alternating with Act only in the tail burst; out DMAs issue on sync
    only (a dma_start blocks its queue's SEQ until the copy resolves).
  - schedule: q-blocks in order 0..3; each block's 16 o_proj tiles are
    queued and interleaved as PE filler into the NEXT block's attention
    (concentrated at diag/normalize points where Act latency would
    otherwise stall the PE). The last block runs as two 256-query halves
    (half the tail overlaps the second half's attention), and its tiles
    borrow the freed attention PSUM banks.
"""

import sys

sys.path.insert(0, "/opt/trn_rl_repo")

import numpy as np
import ml_dtypes

import concourse.bass as bass  # noqa: F401  (bass must import before tile)
import concourse.mybir as mybir
import concourse.tile as tile
from concourse import bacc
from concourse.bass_utils import run_bass_kernel_spmd

N_CORES = 8
T = 2048
HID = 2048
NQ, NKV, D = 16, 8, 128
HQ = NQ // N_CORES  # q heads per core = 2
EPS = 1e-6
SCALE = D**-0.5
SHIFT = 2.0
WS = 64.0   # Wq/Wk pre-scale (cancels in RMS norm)
VS = 32.0   # Wv pre-scale == ctx scale alpha (fp8 range)
BETA = 64.0  # Wo pre-scale (fp8 range); host divides by VS*BETA

P = 128
H = D // 2
KP = HID // 256     # 8 K-pair chunks of 256
NTR = T // 512      # 4 T-ranges of 512

F32 = mybir.dt.float32
BF16 = mybir.dt.bfloat16
F8 = mybir.dt.float8e4
DR = mybir.MatmulPerfMode.DoubleRow
ACT_EXP = mybir.ActivationFunctionType.Exp
ACT_SQRT = mybir.ActivationFunctionType.Sqrt
ACT_SQUARE = mybir.ActivationFunctionType.Square
MIN = mybir.AluOpType.min
MULT = mybir.AluOpType.mult
SUB = mybir.AluOpType.subtract

QR_ORDER = [0, 1, 2, 3]  # last one takes the bf16-copy output path


def build_nc():
    nc = bacc.Bacc("TRN2", target_bir_lowering=False, debug=False,
                   num_devices=N_CORES)

    # ---- DRAM tensors (names = in_map keys) ----
    xh = nc.dram_tensor("xh", [P, KP, 2, T], F8, kind="ExternalInput")
    xl = nc.dram_tensor("xl", [P, KP, 2, T], F8, kind="ExternalInput")
    wqh = nc.dram_tensor("wqh", [P, KP, 2, HQ * D], F8, kind="ExternalInput")
    wql = nc.dram_tensor("wql", [P, KP, 2, HQ * D], F8, kind="ExternalInput")
    wkh = nc.dram_tensor("wkh", [P, KP, 2, D], F8, kind="ExternalInput")
    wkl = nc.dram_tensor("wkl", [P, KP, 2, D], F8, kind="ExternalInput")
    wvh = nc.dram_tensor("wvh", [P, KP, 2, D], F8, kind="ExternalInput")
    wvl = nc.dram_tensor("wvl", [P, KP, 2, D], F8, kind="ExternalInput")
    woh = nc.dram_tensor("woh", [P, HQ, HID], F8, kind="ExternalInput")
    wol = nc.dram_tensor("wol", [P, HQ, HID], F8, kind="ExternalInput")
    cosT = nc.dram_tensor("cosT", [P, T], BF16, kind="ExternalInput")
    sinT = nc.dram_tensor("sinT", [P, T], BF16, kind="ExternalInput")
    qw = nc.dram_tensor("qw", [P, 1], F32, kind="ExternalInput")
    kw = nc.dram_tensor("kw", [P, 1], F32, kind="ExternalInput")
    masks = nc.dram_tensor("masks", [P, P], BF16, kind="ExternalInput")
    out = nc.dram_tensor("out", [T, HID], BF16, kind="ExternalOutput")

    with tile.TileContext(nc) as tc:
        with (
            tc.tile_pool(name="cst", bufs=1) as cst,
            tc.tile_pool(name="fin", bufs=1) as fin,
        ):
            # ---------- constants / weights resident in SBUF ----------
            xh_sb = cst.tile([P, KP, 2, T], F8)
            xl_sb = cst.tile([P, KP, 2, T], F8)
            wqh_sb = cst.tile([P, KP, 2, HQ * D], F8)
            wql_sb = cst.tile([P, KP, 2, HQ * D], F8)
            wkh_sb = cst.tile([P, KP, 2, D], F8)
            wkl_sb = cst.tile([P, KP, 2, D], F8)
            wvh_sb = cst.tile([P, KP, 2, D], F8)
            wvl_sb = cst.tile([P, KP, 2, D], F8)
            woh_sb = cst.tile([P, HQ, HID], F8)
            wol_sb = cst.tile([P, HQ, HID], F8)
            masks_sb = cst.tile([P, P], BF16)
            cos_sb = cst.tile([P, T], BF16)
            sin_sb = cst.tile([P, T], BF16)
            qw_sb = cst.tile([P, 1], F32)
            kw_sb = cst.tile([P, 1], F32)
            nc.scalar.dma_start(qw_sb[:], qw[:])
            nc.scalar.dma_start(kw_sb[:], kw[:])
            ones_b = cst.tile([P, 1], BF16)
            nc.vector.memset(ones_b[:], 1.0)
            # DoubleRow ldweights requires the 2-plane dim step % 16 == 0
            w1_8 = cst.tile([P, 2, 16], F8)
            nc.vector.memset(w1_8[:], 1.0)
            shift_sb = cst.tile([P, 1], F32)
            nc.vector.memset(shift_sb[:], -SHIFT)

            # post RMS+RoPE q/k in bf16 (d on partitions)
            qT = [fin.tile([P, T], BF16, name=f"qT_{s}") for s in range(3)]
            # V (VS x): fp8 plane-pairs (plane = st parity) + bf16 st 0-3
            vp = fin.tile([P, T // 256, 2, D], F8)
            v0b = fin.tile([P, 4, D], BF16)
            # normalized ctx (VS x), fp8 hi/lo, plane = head
            ctxC = fin.tile([P, HQ, T], F8)
            ctxL = fin.tile([P, HQ, T], F8)

            # ==== SBUF pools span both phases (a reopened pool would reuse
            # phase A's region and serialize phase C behind its last reader)
            with (
                tc.tile_pool(name="tmpp", bufs=4) as tmpp,
                tc.tile_pool(name="atp", bufs=6) as atp,
                tc.tile_pool(name="adp", bufs=4) as adp,
                tc.tile_pool(name="cfp", bufs=2) as cfp,
                tc.tile_pool(name="otp", bufs=5) as otp,
                tc.tile_pool(name="ot2p", bufs=4) as ot2p,
                tc.tile_pool(name="attp", bufs=3) as attp,
            ):
              # ==== Phase A (split-fp8 DR projections) + B (RMS+RoPE) ====
              deferred = []
              with (
                tc.tile_pool(name="psA", bufs=4, space="PSUM") as psA,
                tc.tile_pool(name="psV", bufs=2, space="PSUM") as psV,
                tc.tile_pool(name="psB", bufs=2, space="PSUM") as psB,
              ):
                for tr in range(NTR):
                    ts = slice(tr * 512, (tr + 1) * 512)
                    if tr == 0:
                        # x is fully SBUF-resident. All phase-A-critical
                        # loads go through ONE queue (SP) in exact need
                        # order -- DMA_ENGINES serves transfers in DGE
                        # arrival order, so multiple queues let late bulk
                        # loads cut ahead of soon-needed weights. The tr0
                        # chunks are kp-split so the PE can start early.
                        nc.sync.dma_start(wqh_sb[:], wqh[:])
                        nc.sync.dma_start(xh_sb[:, 0:4, :, 0:512],
                                          xh[:, 0:4, :, 0:512])
                        nc.sync.dma_start(xh_sb[:, 4:8, :, 0:512],
                                          xh[:, 4:8, :, 0:512])
                        nc.sync.dma_start(wkh_sb[:], wkh[:])
                        nc.sync.dma_start(wvh_sb[:], wvh[:])
                        nc.sync.dma_start(wql_sb[:], wql[:])
                        nc.sync.dma_start(wkl_sb[:], wkl[:])
                        nc.sync.dma_start(wvl_sb[:], wvl[:])
                        nc.sync.dma_start(xl_sb[:, 0:4, :, 0:512],
                                          xl[:, 0:4, :, 0:512])
                        nc.sync.dma_start(xl_sb[:, 4:8, :, 0:512],
                                          xl[:, 4:8, :, 0:512])
                        for r in range(1, NTR):
                            rs = slice(r * 512, (r + 1) * 512)
                            nc.sync.dma_start(xh_sb[:, :, :, rs],
                                              xh[:, :, :, rs])
                            nc.sync.dma_start(xl_sb[:, :, :, rs],
                                              xl[:, :, :, rs])
                        # cos/sin at the tail of the SP stream: program
                        # order precedes tr0's RoPE reads (required for dep
                        # tracking) but the transfers queue after the x
                        # stream (RoPE tolerates late cos/sin)
                        nc.sync.dma_start(cos_sb[:], cosT[:])
                        nc.sync.dma_start(sin_sb[:], sinT[:])
                    if tr == 2:
                        nc.gpsimd.dma_start(masks_sb[:], masks[:])
                        nc.gpsimd.dma_start(woh_sb[:], woh[:])
                        nc.gpsimd.dma_start(wol_sb[:], wol[:])

                    terms = ((wqh_sb, wkh_sb, wvh_sb, xh_sb),
                             (wql_sb, wkl_sb, wvl_sb, xh_sb),
                             (wqh_sb, wkh_sb, wvh_sb, xl_sb))

                    # --- projections q0, q1, k + direct-transposed v ---
                    # tr 0 runs term-outer so the lo-weight / x-lo DMAs are
                    # needed as late as possible while the stream warms up;
                    # later trs run s-outer (one psA tile in flight each)
                    psv = psV.tile([P, 4, D], F32, name="psv")

                    def v_mm(wv_t, xt_, kp, j, first, last):
                        jts = slice(tr * 512 + j * P,
                                    tr * 512 + (j + 1) * P)
                        nc.tensor.matmul(
                            psv[:, j, :], xt_[:, kp, :, jts],
                            wv_t[:, kp, :, :],
                            perf_mode=DR, start=first, stop=last,
                        )

                    def qk_mm(ps, s, wq_t, wk_t, xt_, kp, first, last):
                        wt = wq_t if s < 2 else wk_t
                        cs = slice(s * D, (s + 1) * D) if s < 2 \
                            else slice(0, D)
                        nc.tensor.matmul(
                            ps[:], wt[:, kp, :, cs], xt_[:, kp, :, ts],
                            perf_mode=DR, start=first, stop=last,
                        )

                    if tr == 0:
                        raw = [psA.tile([P, 512], F32, name="psA_t")
                               for _ in range(3)]
                        for t_, (wq_t, wk_t, wv_t, xt_) in enumerate(terms):
                            for kp in range(KP):
                                first = t_ == 0 and kp == 0
                                last = t_ == 2 and kp == KP - 1
                                for s in range(3):
                                    qk_mm(raw[s], s, wq_t, wk_t, xt_,
                                          kp, first, last)
                    else:
                        raw = []
                        for s in range(3):
                            ps = psA.tile([P, 512], F32, name="psA_t")
                            i = 0
                            for wq_t, wk_t, _, xt_ in terms:
                                for kp in range(KP):
                                    qk_mm(ps, s, wq_t, wk_t, xt_, kp,
                                          i == 0, i == 3 * KP - 1)
                                    i += 1
                            raw.append(ps)
                    # one accumulation group for the whole bank: the
                    # first start lazily zeroes the full 2KB zero region
                    for j in range(4):
                        i = 0
                        for _, _, wv_t, xt_ in terms:
                            for kp in range(KP):
                                v_mm(wv_t, xt_, kp, j,
                                     j == 0 and i == 0,
                                     j == 3 and i == 3 * KP - 1)
                                i += 1
                    for j in range(4):
                        st = 4 * tr + j
                        nc.vector.tensor_copy(vp[:, st // 2, st % 2, :],
                                              psv[:, j, :])
                        if tr == 0:
                            nc.vector.tensor_copy(v0b[:, st, :], psv[:, j, :])

                    # --- B: RMS norm + RoPE for q0, q1, k (bf16) ---
                    # tr3's DVE/Pool back-half (bcast+stt+RoPE) is deferred
                    # past qr0's attention so phase C's DVE work isn't
                    # queued behind it (qr3 needs tr3's qT much later)
                    for s in range(3):
                        w_sb = qw_sb if s < 2 else kw_sb
                        # free the psA bank early for the next projection /
                        # the phase-C PSUM pools (Pool has slack here)
                        src = tmpp.tile([P, 512], F32, name="src")
                        nc.scalar.copy(src[:], raw[s][:])
                        sq = tmpp.tile([P, 512], BF16, name="sq")
                        nc.scalar.activation(sq[:], src[:], ACT_SQUARE)
                        ssum = psB.tile([1, 512], F32, name="ssum")
                        nc.tensor.matmul(ssum[:], ones_b[:], sq[:],
                                         start=True, stop=True)
                        # src holds 64*q; host folds sqrt(D) into q/k norm
                        # weights, eps is negligible vs ssum ~ 3e5
                        rstd = tmpp.tile([1, 512], F32, name="rstd")
                        nc.scalar.activation(rstd[:], ssum[:], ACT_SQRT)
                        rinv = tmpp.tile([1, 512], F32, name="rinv")
                        nc.vector.reciprocal_approx_fast(rinv[:], rstd[:])

                        def back_half(s=s, w_sb=w_sb, src=src, rinv=rinv,
                                      ts=ts):
                            rb = tmpp.tile([P, 512], F32, name="rb")
                            nc.gpsimd.partition_broadcast(rb[:], rinv[:])
                            nq = tmpp.tile([P, 512], BF16, name="nq")
                            nc.vector.scalar_tensor_tensor(
                                nq[:], src[:], w_sb[:], rb[:], MULT, MULT,
                            )
                            # RoPE: sin pre-rolled by 64 partitions with the
                            # rotate-half sign folded in; one full-width add
                            psn = tmpp.tile([P, 512], BF16, name="psn")
                            nc.vector.tensor_mul(psn[0:H, :], nq[H:D, :],
                                                 sin_sb[H:D, ts])
                            nc.vector.tensor_mul(psn[H:D, :], nq[0:H, :],
                                                 sin_sb[0:H, ts])
                            pc = tmpp.tile([P, 512], BF16, name="pc")
                            nc.vector.tensor_mul(pc[:], nq[:], cos_sb[:, ts])
                            nc.vector.tensor_add(qT[s][:, ts], pc[:], psn[:])

                        if tr == NTR - 1:
                            deferred.append(back_half)
                        else:
                            back_half()

              # ===== Phase C: attention + o_proj =====
              with (
                tc.tile_pool(name="psP", bufs=2, space="PSUM") as psP,
                tc.tile_pool(name="psCX", bufs=2, space="PSUM") as psCX,
                tc.tile_pool(name="psSM", bufs=1, space="PSUM") as psSM,
                tc.tile_pool(name="psD", bufs=3, space="PSUM") as psD,
              ):
                kT = qT[2]
                pending = []
                tail_dmas = []
                pair_stash = {}

                def make_task(qr, tt, nr, idx, pools=None, tail=False,
                              dma_list=None):
                    abs_tt = 4 * qr + tt
                    tts = slice(abs_tt * P, (abs_tt + 1) * P)
                    ns = slice(nr * 512, (nr + 1) * 512)

                    def go():
                        if pools is None:
                            ps = psD.tile([P, 512], F32, name="psD_t")
                        else:
                            # tail: attention PSUM banks are free; borrow
                            # them so the last o_proj burst isn't throttled
                            # by psD recycling
                            pool, tag = pools[idx % len(pools)]
                            ps = pool.tile([P, 512], F32, name=tag)
                        for i, (cs, ws) in enumerate(
                                ((ctxC, woh_sb), (ctxL, woh_sb),
                                 (ctxC, wol_sb))):
                            nc.tensor.matmul(
                                ps[:], cs[:, :, tts], ws[:, :, ns],
                                perf_mode=DR,
                                start=(i == 0), stop=(i == 2))
                        if dma_list is not None:
                            # tail burst: adjacent tiles share a [P,1024]
                            # staging tile (halves the serially-issued
                            # HWDGE DMA count); DMAs go in a second pass
                            # on two queues after the copies are in flight
                            key = (tt, nr // 2)
                            if nr % 2 == 0:
                                ot2 = ot2p.tile([P, 1024], BF16,
                                                name="ot2")
                                pair_stash[key] = ot2
                            else:
                                ot2 = pair_stash.pop(key)
                            osl = ot2[:, (nr % 2) * 512:(nr % 2 + 1) * 512]
                            if idx % 2 == 0:
                                nc.scalar.copy(osl, ps[:])
                            else:
                                nc.vector.tensor_copy(osl, ps[:])
                            if nr % 2 == 1:
                                ns2 = slice((nr // 2) * 1024,
                                            (nr // 2 + 1) * 1024)
                                dma_list.append((ot2, tts, ns2))
                        else:
                            ot = otp.tile([P, 512], BF16, name="ot")
                            # GPSIMD cannot read PSUM; copies go DVE
                            # during attention (Act is exp-bound)
                            if tail and idx % 2 == 0:
                                nc.scalar.copy(ot[:], ps[:])
                            else:
                                nc.vector.tensor_copy(ot[:], ps[:])
                            # out DMAs only on sync: a dma_start blocks
                            # its issuing engine's SEQ until the copy
                            # dependency resolves, so compute queues must
                            # not carry them
                            nc.sync.dma_start(out[tts, ns], ot[:])
                    return go

                def emit_fill(k):
                    for _ in range(min(k, len(pending))):
                        pending.pop(0)()

                def attn_part(qr, h, qoff, qlen):
                    """Attention for queries [qr*512+qoff, +qlen) of head h."""
                    qs = slice(qr * 512 + qoff, qr * 512 + qoff + qlen)
                    at_dt = BF16 if qr == 0 else F8
                    n_off = 2 * qr
                    ctx_ps = psCX.tile([P, 512], F32, name="ctx_ps")
                    sums_t = psSM.tile([1, 512], F32, name="sums_t")
                    sums_ps = sums_t[:]
                    j0, j1 = qoff // P, (qoff + qlen) // P
                    # --- fully-causal pairs below the diagonal block ---
                    for pi in range(n_off):
                        at = atp.tile([P, 2, 512], at_dt, name="at")
                        for half in range(2):
                            st = 2 * pi + half
                            s_ps = psP.tile([P, 512], F32, name="s_t")
                            nc.tensor.matmul(
                                s_ps[:, 0:qlen], kT[:, st * P:(st + 1) * P],
                                qT[h][:, qs], start=True, stop=True)
                            nc.scalar.activation(
                                at[:, half, 0:qlen], s_ps[:, 0:qlen],
                                ACT_EXP, scale=SCALE, bias=shift_sb[:])
                        nc.tensor.matmul(
                            ctx_ps[:, 0:qlen], vp[:, pi, :, :],
                            at[:, :, 0:qlen],
                            perf_mode=DR, start=(pi == 0), stop=False)
                        nc.tensor.matmul(
                            sums_ps[:, 0:qlen], w1_8[:, :, 0:1],
                            at[:, :, 0:qlen],
                            perf_mode=DR, start=(pi == 0), stop=False)
                    # --- diagonal block, 128-query granular ---
                    for j in range(j0, j1):
                        emit_fill(1)
                        jr = j - j0
                        jsl = slice(jr * P, (jr + 1) * P)
                        qjs = slice(qr * 512 + j * P, qr * 512 + (j + 1) * P)
                        sd = psP.tile([P, 4, P], F32, name="s_t")
                        for i in range(j + 1):
                            st = 4 * qr + i
                            nc.tensor.matmul(
                                sd[:, i, :], kT[:, st * P:(st + 1) * P],
                                qT[h][:, qjs], start=True, stop=True)
                        ad = adp.tile([P, 4, P], at_dt, name="ad")
                        nc.scalar.activation(
                            ad[:, 0:j + 1, :], sd[:, 0:j + 1, :],
                            ACT_EXP, scale=SCALE, bias=shift_sb[:])
                        # only the true-diagonal tile needs masking
                        nc.vector.tensor_tensor(
                            ad[:, j, :], ad[:, j, :], masks_sb[:], MIN)
                        if qr == 0:
                            # one group: first start zeroes the whole bank
                            for i in range(j + 1):
                                st_ = j == j0 and i == 0
                                fin0 = j == j1 - 1 and i == j
                                nc.tensor.matmul(
                                    ctx_ps[:, jsl], v0b[:, i, :],
                                    ad[:, i, :], start=st_, stop=fin0)
                                nc.tensor.matmul(
                                    sums_ps[:, jsl], ones_b[:],
                                    ad[:, i, :], start=st_, stop=fin0)
                        else:
                            # one group (opened by off-diag pi==0 over the
                            # full query range): stop only on the final
                            # matmul of the last diag subtile
                            fin = j == j1 - 1
                            np_full = (j + 1) // 2
                            for p_ in range(np_full):
                                last_ = (j % 2 == 1) and (p_ == np_full - 1)
                                nc.tensor.matmul(
                                    ctx_ps[:, jsl],
                                    vp[:, 2 * qr + p_, :, :],
                                    ad[:, 2 * p_:2 * p_ + 2, :],
                                    perf_mode=DR, start=False,
                                    stop=(fin and last_))
                                nc.tensor.matmul(
                                    sums_ps[:, jsl], w1_8[:, :, 0:1],
                                    ad[:, 2 * p_:2 * p_ + 2, :],
                                    perf_mode=DR, start=False,
                                    stop=(fin and last_))
                            if j % 2 == 0:  # odd plane count: tail tile
                                nc.tensor.matmul(
                                    ctx_ps[:, jsl],
                                    vp[:, 2 * qr + j // 2, j % 2, :],
                                    ad[:, j, :], start=False, stop=fin)
                                nc.tensor.matmul(
                                    sums_ps[:, jsl], w1_8[:, 0, 0:1],
                                    ad[:, j, :], start=False, stop=fin)
                    # --- normalize + fp8 hi/lo split of ctx ---
                    recip = attp.tile([1, 512], F32, name="recip")
                    nc.vector.reciprocal_approx_fast(
                        recip[:, 0:qlen], sums_ps[:, 0:qlen])
                    rb = attp.tile([P, 512], F32, name="rbc")
                    nc.gpsimd.partition_broadcast(
                        rb[:, 0:qlen], recip[:, 0:qlen])
                    cf = cfp.tile([P, 512], F32, name="cf")
                    nc.vector.tensor_mul(
                        cf[:, 0:qlen], ctx_ps[:, 0:qlen], rb[:, 0:qlen])
                    nc.gpsimd.tensor_copy(ctxC[:, h, qs], cf[:, 0:qlen])
                    nc.vector.scalar_tensor_tensor(
                        ctxL[:, h, qs], cf[:, 0:qlen], 1.0, ctxC[:, h, qs],
                        MULT, SUB)
                    emit_fill(3)

                for qi, qr in enumerate(QR_ORDER):
                    last = qi == len(QR_ORDER) - 1
                    # the last block runs as two query halves so half of
                    # its o_proj overlaps the second half's attention
                    parts = ((0, 256), (256, 256)) if last else ((0, 512),)
                    for qoff, qlen in parts:
                        for h in range(HQ):
                            attn_part(qr, h, qoff, qlen)
                        fin_part = last and qoff > 0
                        tailp = [(psD, "psD_t"), (psP, "s_t"),
                                 (psCX, "ctx_ps")] if fin_part else None
                        dml = tail_dmas if fin_part else None
                        for tt in range(qoff // P, (qoff + qlen) // P):
                            for nr in range(4):
                                pending.append(
                                    make_task(qr, tt, nr, 4 * tt + nr,
                                              pools=tailp, tail=last,
                                              dma_list=dml))
                    if qi == 0:
                        for fn in deferred:
                            fn()
                        deferred.clear()
                    if last:
                        emit_fill(len(pending))
                        for i, (ot, tts, ns) in enumerate(tail_dmas):
                            eng = nc.sync if i % 2 == 0 else nc.scalar
                            eng.dma_start(out[tts, ns], ot[:])

    nc.compile()
    return nc


_NC_CACHE = None


def get_nc():
    global _NC_CACHE
    if _NC_CACHE is None:
        _NC_CACHE = build_nc()
    return _NC_CACHE


F8NP = ml_dtypes.float8_e4m3
BF16NP = ml_dtypes.bfloat16


def _fold_hid(a):
    """[HID, C] -> [P, KP, 2, C] with hid = kp*256 + pl*128 + p."""
    c = a.shape[1]
    return np.ascontiguousarray(
        a.reshape(KP, 2, P, c).transpose(2, 0, 1, 3))


def _split8(a):
    hi = a.astype(F8NP)
    lo = (a - hi.astype(np.float32)).astype(F8NP)
    return hi, lo


def make_in_maps(x, cos, sin, Wq, Wk, Wv, Wo, q_norm_w, k_norm_w):
    x = np.asarray(x, dtype=np.float32).reshape(T, HID)
    xf = _fold_hid(np.ascontiguousarray(x.T).reshape(HID, T))
    xh, xl = _split8(xf)
    cosb = np.ascontiguousarray(
        np.asarray(cos, np.float32).T).astype(BF16NP)
    # rolled by 64 with rotate-half signs folded in:
    # psn[0:64] (subtracted in ref) uses rows 64:128 -> negate those rows
    sr = np.roll(np.asarray(sin, np.float32).T, 64, axis=0)
    sr[64:, :] *= -1.0
    sinb = np.ascontiguousarray(sr).astype(BF16NP)
    # sqrt(D) folded here: kernel computes rinv = (sum (64 q)^2)^-0.5
    sqd = np.float32(np.sqrt(D))
    qwa = np.ascontiguousarray(
        np.asarray(q_norm_w, np.float32).reshape(D, 1) * sqd)
    kwa = np.ascontiguousarray(
        np.asarray(k_norm_w, np.float32).reshape(D, 1) * sqd)
    si = np.arange(P)[:, None]
    qi = np.arange(P)[None, :]
    masks = np.where(si <= qi, 240.0, 0.0).astype(BF16NP)
    Wq = np.asarray(Wq, np.float32) * WS
    Wk = np.asarray(Wk, np.float32) * WS
    Wv = np.asarray(Wv, np.float32) * VS
    Wo = np.asarray(Wo, np.float32) * BETA
    in_maps = []
    for c in range(N_CORES):
        wqh_, wql_ = _split8(_fold_hid(Wq[:, c * HQ * D:(c + 1) * HQ * D]))
        wkh_, wkl_ = _split8(_fold_hid(Wk[:, c * D:(c + 1) * D]))
        wvh_, wvl_ = _split8(_fold_hid(Wv[:, c * D:(c + 1) * D]))
        wo_ = np.ascontiguousarray(
            Wo[c * HQ * D:(c + 1) * HQ * D, :].reshape(HQ, P, HID)
            .transpose(1, 0, 2))
        woh_, wol_ = _split8(wo_)
        in_maps.append({
            "xh": xh, "xl": xl,
            "wqh": wqh_, "wql": wql_,
            "wkh": wkh_, "wkl": wkl_,
            "wvh": wvh_, "wvl": wvl_,
            "woh": woh_, "wol": wol_,
            "cosT": cosb, "sinT": sinb,
            "qw": qwa, "kw": kwa,
            "masks": masks,
        })
    return in_maps


def kernel(x, cos, sin, Wq, Wk, Wv, Wo, q_norm_w, k_norm_w):
    nc = get_nc()
    in_maps = make_in_maps(x, cos, sin, Wq, Wk, Wv, Wo, q_norm_w, k_norm_w)
    res = run_bass_kernel_spmd(nc, in_maps, core_ids=list(range(N_CORES)))
    acc = np.zeros((T, HID), dtype=np.float32)
    for c in range(N_CORES):
        acc += np.asarray(res.results[c]["out"], np.float32)
    acc *= 1.0 / (VS * BETA)
    return acc.reshape(1, T, HID)
